# revision 13
# baseline (speedup 1.0000x reference)
"""Trainium2 Bass kernel for nn_BEE_Bin2Symbol (hyper-decoder + masked-conv
autoregressive MLP decoder).

Architecture (v2, latency-oriented):
- Sequential phase runs all GEMMs in [C_out-on-partitions, B-pixels-free]
  orientation (weights stationary as lhsT, activations moving): matmul cost
  scales with B<=16, transposes disappear, and each layer's nonlinearity is a
  single DVE scalar_tensor_tensor op  max(0.01*z, z)  reading PSUM directly.
- 140 slope-3 anti-diagonal wavefronts. Per step the critical chain is
  fresh-tap MMs -> lrelu(z0) -> W1 MMs -> lrelu(z1) -> ... -> z5 -> Y-add.
  Everything else (old-tap ctx GEMMs, f-part, bias seeds, phase-P conv
  streaming) is issued into the PE gaps between the chain's engine hops.
- Fresh taps (age-1) use precomposed G = W0c @ Wd_tap so they feed z0
  directly; old taps accumulate a ctx PSUM a step ahead, evicted by the
  scalar engine (ACT) off the critical path.
- All sequential-phase weights/activations in bf16 (PSUM accumulate f32).
- Hyper-decoder (2 stride-2 deconvs + 3x3 conv) in f32r as phase-decomposed
  GEMMs; conv2's last 3 row-blocks stream into the first ~15 wavefront steps.
"""
import sys, os
sys.path.insert(0, "/opt/trn_rl_repo")

import numpy as np

import concourse.bass as bass
import concourse.bacc as bacc
import concourse.mybir as mybir
import concourse.tile as tile
from concourse.masks import make_identity

F32 = mybir.dt.float32
F32R = mybir.dt.float32r
BF16 = mybir.dt.bfloat16

H, W = 32, 48
HP, WP = H + 4, W + 4            # padded image 36 x 52
NPIX = H * W
NSTEPS = 3 * (H - 1) + (W - 1) + 1   # 140
DIAG = WP - 3                    # 49: wavefront-diagonal stride in padded img

# taps (dy, dx): source pixel = (i-dy, j-dx); ctx_w index (ky,kx) = (2-dy, 2-dx)
TAPS = [(2, 2), (2, 1), (2, 0), (2, -1), (2, -2),
        (1, 2), (1, 1), (1, 0), (1, -1), (1, -2),
        (0, 1), (0, 2)]
FRESH_TAPS = [(1, -2), (0, 1)]                # age-1 taps (need step t-1)
OLD_TAPS = [d for d in TAPS if d not in FRESH_TAPS]

DIMS = [768, 640, 512, 384, 320, 256, 192]   # MLP dims; layer l: DIMS[l]->DIMS[l+1]
# z1..z5 chunk offsets inside the shared zs psum tile [128, 14, 16]
ZOFF = {1: 0, 2: 4, 3: 7, 4: 10, 5: 12}


def cdiv(a, b):
    return (a + b - 1) // b


def chunks_of(n, c=128):
    return [(s, min(c, n - s)) for s in range(0, n, c)]


def _ap(tile_ap, slot_off, elem_off, plist):
    """Build a custom AP into a [128, S, F]-shaped sbuf/psum tile."""
    base = tile_ap[:]
    return bass.AP(base.tensor, base.offset + slot_off + elem_off, plist)


def step_geom(t):
    i_lo = max(0, cdiv(t - (W - 1), 3))
    i_hi = min(H - 1, t // 3)
    return i_lo, i_hi - i_lo + 1, t - 3 * i_lo


def build(nsteps=NSTEPS):
    nc = bacc.Bacc()

    # ---------------- DRAM I/O ----------------
    di = {}
    di['z_hat'] = nc.dram_tensor('z_hat', [1, 192, 8, 12], F32, kind="ExternalInput")
    di['w_hat'] = nc.dram_tensor('w_hat', [1, 192, 32, 48], F32, kind="ExternalInput")
    di['hs_dw0'] = nc.dram_tensor('hs_dw0', [192, 192, 5, 5], F32, kind="ExternalInput")
    di['hs_db0'] = nc.dram_tensor('hs_db0', [192], F32, kind="ExternalInput")
    di['hs_dw1'] = nc.dram_tensor('hs_dw1', [192, 288, 5, 5], F32, kind="ExternalInput")
    di['hs_db1'] = nc.dram_tensor('hs_db1', [288], F32, kind="ExternalInput")
    di['hs_cw2'] = nc.dram_tensor('hs_cw2', [384, 288, 3, 3], F32, kind="ExternalInput")
    di['hs_cb2'] = nc.dram_tensor('hs_cb2', [384], F32, kind="ExternalInput")
    di['ctx_w'] = nc.dram_tensor('ctx_w', [384, 192, 5, 5], F32, kind="ExternalInput")
    di['ctx_b'] = nc.dram_tensor('ctx_b', [384], F32, kind="ExternalInput")
    for li in range(6):
        di[f'ep_w{li}'] = nc.dram_tensor(f'ep_w{li}', [DIMS[li + 1], DIMS[li]], F32,
                                         kind="ExternalInput")
        di[f'ep_b{li}'] = nc.dram_tensor(f'ep_b{li}', [DIMS[li + 1]], F32,
                                         kind="ExternalInput")
    out = nc.dram_tensor('out', [1, 192, 32, 48], F32, kind="ExternalOutput")

    with tile.TileContext(nc) as tc:
        with tc.tile_pool(name="pp", bufs=1) as pp, \
             tc.tile_pool(name="pps", bufs=1, space="PSUM") as pps:

            ident = pp.tile([128, 128], F32)
            make_identity(nc, ident[:])

            # ---------- persistent state ----------
            Yimg = pp.tile([128, 2, HP * WP], BF16)   # decoded image (padded)
            nc.gpsimd.memset(Yimg[:], 0.0)
            wimg = pp.tile([128, 2, HP * WP], F32)    # w_hat residual (padded)
            nc.gpsimd.memset(wimg[:], 0.0)
            fm1 = pp.tile([128, 3, NPIX], BF16)       # conv2 output [384, 1536]
            m2 = pp.tile([128, 3, 34 * 50], BF16)     # deconv1 out (padded 34x50)
            nc.gpsimd.memset(m2[:], 0.0)
            cw2T = pp.tile([128, 3, 3, 9 * 128], BF16)  # [cin, mi, si, k*128+o]

            # transposed weights (bf16)
            W0fT = pp.tile([128, 3, 640], BF16)
            W0cT = pp.tile([128, 3, 640], BF16)
            WT = {}
            for li in range(1, 6):
                WT[li] = pp.tile([128, cdiv(DIMS[li], 128), DIMS[li + 1]], BF16,
                                 tag=f"W{li}T", name=f"W{li}T")
            GT = [pp.tile([128, 2, 640], BF16, tag=f"GT{k}", name=f"GT{k}") for k in range(2)]
            WdT = {}
            for d in OLD_TAPS:
                WdT[d] = pp.tile([128, 2, 384], BF16, tag=f"Wd{d[0]}_{d[1]}", name=f"Wd{d[0]}_{d[1]}")

            # bias rows (lhsT for K=1 seed matmuls)
            brow = [pp.tile([1, DIMS[li + 1]], BF16, tag=f"b{li}", name=f"b{li}") for li in range(6)]
            ctxb = pp.tile([1, 384], BF16)
            ones = pp.tile([1, 16], BF16)
            nc.vector.memset(ones[:], 1.0)

            # sequential-phase activations (single tiles; WAR handled by sems)
            xs = {li: pp.tile([128, cdiv(DIMS[li], 128), 16], BF16, tag=f"x{li}", name=f"x{li}")
                  for li in range(1, 6)}
            Xc = pp.tile([128, 3, 16], BF16)          # evicted old-ctx

            # persistent psum: z0+ctx ring (2 banks), mlp zs (1), conv stream (1)
            zc = [pps.tile([128, 8, 16], F32, tag=f"zc{s}", name=f"zc{s}") for s in range(2)]
            zs = pps.tile([128, 14, 16], F32, tag="zs")
            nc.vector.memset(zs[:], 0.0)
            nc.vector.memset(zc[0][:], 0.0)
            nc.vector.memset(zc[1][:], 0.0)

            # ================= PROLOGUE =================
            with tc.tile_pool(name="pro", bufs=2) as pro, \
                 tc.tile_pool(name="prps", bufs=2, space="PSUM") as prps:

                # --- w_hat into padded residual image ---
                whv = di['w_hat'].ap()[0]
                for ci, (s, cw) in enumerate(chunks_of(192)):
                    dst = _ap(wimg, ci * HP * WP, 2 * WP + 2,
                              [[2 * HP * WP, cw], [WP, H], [1, W]])
                    nc.sync.dma_start(dst, whv[s:s + cw])

                # --- bias rows ---
                def load_brow(dst, dram, n):
                    st = pro.tile([1, 640], F32, tag="brs", name="brs")
                    nc.sync.dma_start(st[0:1, 0:n], dram.ap()[None, :])
                    nc.vector.tensor_copy(dst[0:1, 0:n], st[0:1, 0:n])
                for li in range(6):
                    load_brow(brow[li], di[f'ep_b{li}'], DIMS[li + 1])
                load_brow(ctxb, di['ctx_b'], 384)

                # --- transpose ep_w0..5 -> bf16 (alternate DVE/ACT for evicts) ---
                W0cT32 = pro.tile([128, 3, 640], F32, tag="w0c32", bufs=1)
                tp_count = [0]
                def evict(dst_ap, src_ap):
                    if tp_count[0] % 2 == 0:
                        nc.vector.tensor_copy(dst_ap, src_ap)
                    else:
                        nc.scalar.activation(dst_ap, src_ap,
                                             mybir.ActivationFunctionType.Copy)
                    tp_count[0] += 1

                def load_and_transpose(li):
                    n_out, n_in = DIMS[li + 1], DIMS[li]
                    wnat = pro.tile([128, 6, 768], F32, tag="wnat", bufs=1)
                    for mi, (ms, mw) in enumerate(chunks_of(n_out)):
                        nc.sync.dma_start(wnat[0:mw, mi, 0:n_in],
                                          di[f'ep_w{li}'].ap()[ms:ms + mw])
                    for ci, (cs, cww) in enumerate(chunks_of(n_in)):
                        for mi, (ms, mw) in enumerate(chunks_of(n_out)):
                            pt = prps.tile([128, 128], F32, tag="tp")
                            nc.tensor.transpose(pt[0:cww, 0:mw],
                                                wnat[0:mw, mi, cs:cs + cww],
                                                ident[0:mw, 0:mw])
                            if li == 0:
                                if ci < 3:
                                    evict(W0fT[0:cww, ci, ms:ms + mw], pt[0:cww, 0:mw])
                                else:
                                    # ctx half: need bf16 AND f32 (for G compose)
                                    nc.vector.tensor_copy(
                                        W0cT[0:cww, ci - 3, ms:ms + mw], pt[0:cww, 0:mw])
                                    nc.scalar.activation(
                                        W0cT32[0:cww, ci - 3, ms:ms + mw], pt[0:cww, 0:mw],
                                        mybir.ActivationFunctionType.Copy)
                            else:
                                evict(WT[li][0:cww, ci, ms:ms + mw], pt[0:cww, 0:mw])
                for li in range(6):
                    load_and_transpose(li)

                # --- ctx_w rows; WdT (old taps) via transpose; GT (fresh) compose ---
                cwn = [pro.tile([128, 192 * 25], F32, tag=f"cwn{mi}", bufs=1, name=f"cwn{mi}")
                       for mi in range(3)]
                for mi in range(3):
                    nc.sync.dma_start(
                        cwn[mi][:],
                        di['ctx_w'].ap()[mi * 128:(mi + 1) * 128]
                        .rearrange("o c kh kw -> o (c kh kw)"))
                for d in OLD_TAPS:
                    ky, kx = 2 - d[0], 2 - d[1]
                    for mi in range(3):
                        for ci, (cs, cww) in enumerate(chunks_of(192)):
                            src = _ap(cwn[mi], 0, cs * 25 + ky * 5 + kx,
                                      [[192 * 25, 128], [25, cww]])
                            pt = prps.tile([128, 128], F32, tag="tp")
                            nc.tensor.transpose(pt[0:cww, 0:128], src, ident[:])
                            evict(WdT[d][0:cww, ci, mi * 128:(mi + 1) * 128],
                                  pt[0:cww, 0:128])
                # GT[f] = (W0c @ Wd_tap)^T = Wd^T W0c^T : out [192, 640]
                for f, d in enumerate(FRESH_TAPS):
                    ky, kx = 2 - d[0], 2 - d[1]
                    for mc, (cs, cww) in enumerate(chunks_of(192)):
                        for nh in range(2):   # 640 -> 2 x 320
                            gp = prps.tile([128, 384], F32, tag="dps")
                            for ki in range(3):
                                lhsT = _ap(cwn[ki], 0, cs * 25 + ky * 5 + kx,
                                           [[192 * 25, 128], [25, cww]]).bitcast(F32R)
                                rhs = W0cT32[0:128, ki, nh * 320:(nh + 1) * 320].bitcast(F32R)
                                nc.tensor.matmul(gp[0:cww, 0:320], lhsT, rhs,
                                                 start=(ki == 0), stop=(ki == 2),
                                                 skip_group_check=True)
                            evict(GT[f][0:cww, mc, nh * 320:(nh + 1) * 320],
                                  gp[0:cww, 0:320])

            # ============ PROLOGUE B: hyper-decoder ============
            with tc.tile_pool(name="proB", bufs=2) as pro, \
                 tc.tile_pool(name="prpsB", bufs=2, space="PSUM") as prps:

                # --- deconv stack (f32r, phase-decomposed) ---
                def load_bias_col(name, n):
                    nch = cdiv(n, 128)
                    t = pp.tile([128, nch], F32, tag=f"b_{name}", name=f"b_{name}")
                    nc.vector.memset(t[:], 0.0)
                    for ci, (s, w_) in enumerate(chunks_of(n)):
                        nc.sync.dma_start(t[0:w_, ci:ci + 1], di[name][s:s + w_][:, None])
                    return t
                b_d0 = load_bias_col('hs_db0', 192)
                b_d1 = load_bias_col('hs_db1', 288)
                b_c2 = load_bias_col('hs_cb2', 384)

                def deconv_chunk(inp_t, inp_hw, w_t, cin, mw, mi, out_t, bias_t):
                    hi, wi = inp_hw
                    ip_w = wi + 2
                    op_w = 2 * wi + 2
                    for py in range(2):
                        for px in range(2):
                            ps = prps.tile([128, 16 * 24], F32, tag="dps")
                            first = True
                            taps = [(u, v) for u in range(py, 5, 2) for v in range(px, 5, 2)]
                            for ti, (u, v) in enumerate(taps):
                                dy = (py + 2 - u) // 2
                                dx = (px + 2 - v) // 2
                                for ci in range(cdiv(cin, 128)):
                                    lhsT = _ap(w_t, ci * 128 * 25, u * 5 + v,
                                               [[2 * 128 * 25, 128], [25, mw]])
                                    rhs = _ap(inp_t, ci * (hi + 2) * ip_w,
                                              (1 + dy) * ip_w + (1 + dx),
                                              [[2 * (hi + 2) * ip_w, 128], [ip_w, hi], [1, wi]])
                                    last = (ti == len(taps) - 1) and (ci == cdiv(cin, 128) - 1)
                                    nc.tensor.matmul(ps[0:mw, 0:hi * wi], lhsT, rhs,
                                                     start=first, stop=last,
                                                     skip_group_check=True)
                                    first = False
                            dst = _ap(out_t, mi * (2 * hi + 2) * op_w,
                                      (py + 1) * op_w + (px + 1),
                                      [[out_t.shape[1] * (2 * hi + 2) * op_w, mw],
                                       [2 * op_w, hi], [2, wi]])
                            nc.scalar.activation(
                                dst, ps[0:mw, 0:hi * wi].rearrange("p (a b) -> p a b", a=hi),
                                mybir.ActivationFunctionType.Lrelu,
                                bias=bias_t[0:mw, mi][:, None], alpha=0.01)

                # deconv0: z[192,8,12] -> m1[192,16,24]
                m1 = pro.tile([128, 2, 18 * 26], F32R, tag="m1", bufs=1)
                nc.gpsimd.memset(m1[:].bitcast(F32), 0.0)
                zp = pro.tile([128, 2, 10 * 14], F32R, tag="zp", bufs=1)
                nc.gpsimd.memset(zp[:].bitcast(F32), 0.0)
                zv = di['z_hat'].ap()[0]
                for ci, (s, cw) in enumerate(chunks_of(192)):
                    dst = _ap(zp, ci * 140, 14 + 1, [[2 * 140, cw], [14, 8], [1, 12]])
                    nc.sync.dma_start(dst.bitcast(F32), zv[s:s + cw])
                for mi, (ms, mw) in enumerate(chunks_of(192)):
                    dw = pro.tile([128, 2, 128 * 25], F32R, tag="dw")
                    nc.gpsimd.memset(dw[64:128, 1, :].bitcast(F32), 0.0)
                    for ci, (cs, cww) in enumerate(chunks_of(192)):
                        nc.sync.dma_start(
                            dw[0:cww, ci, 0:mw * 25].bitcast(F32),
                            di['hs_dw0'].ap()[cs:cs + cww, ms:ms + mw]
                            .rearrange("c o kh kw -> c (o kh kw)"))
                    deconv_chunk(zp, (8, 12), dw, 192, mw, mi, m1, b_d0)

                # deconv1: m1[192,16,24] -> m2[288,32,48]
                for mi, (ms, mw) in enumerate(chunks_of(288)):
                    dw = pro.tile([128, 2, 128 * 25], F32R, tag="dw")
                    nc.gpsimd.memset(dw[64:128, 1, :].bitcast(F32), 0.0)
                    for ci, (cs, cww) in enumerate(chunks_of(192)):
                        nc.sync.dma_start(
                            dw[0:cww, ci, 0:mw * 25].bitcast(F32),
                            di['hs_dw1'].ap()[cs:cs + cww, ms:ms + mw]
                            .rearrange("c o kh kw -> c (o kh kw)"))
                    deconv_chunk(m1, (16, 24), dw, 192, mw, mi, m2, b_d1)

                # conv2 weights: cw2T[cin, mi, si, k*128+o] via PE transpose
                for mi in range(3):
                    cw2s = pro.tile([128, 288 * 9], F32, tag="cw2s")
                    nc.sync.dma_start(
                        cw2s[:],
                        di['hs_cw2'].ap()[mi * 128:(mi + 1) * 128]
                        .rearrange("o c kh kw -> o (c kh kw)"))
                    nc.gpsimd.memset(cw2T[32:128, mi, 2, :], 0.0)
                    for k in range(9):
                        for si, (ss, sw) in enumerate(chunks_of(288)):
                            src = _ap(cw2s, 0, ss * 9 + k, [[288 * 9, 128], [9, sw]])
                            pt = prps.tile([128, 128], F32, tag="tp")
                            nc.tensor.transpose(pt[0:sw, 0:128], src, ident[:])
                            evict(_ap(cw2T, (mi * 3 + si) * 9 * 128, k * 128,
                                      [[3 * 3 * 9 * 128, sw], [1, 128]]),
                                  pt[0:sw, 0:128])

                # conv2 row-block 0 (rows 0..7) upfront
                for mi in range(3):
                    emit_conv2_unit(nc, pps, cw2T, m2, fm1, b_c2, mi, 0, 0, 27)

            # ================= SEQUENTIAL PHASE =================
            # conv2 streaming state: remaining units (mi, ch>=1), 27 MMs each
            pf_units = [(mi, ch) for ch in range(1, 4) for mi in range(3)]
            pf_state = {"u": 0, "k": 0, "ps": None}

            def pfill(nmm):
                while nmm > 0 and pf_state["u"] < len(pf_units):
                    mi, ch = pf_units[pf_state["u"]]
                    take = min(nmm, 27 - pf_state["k"])
                    ps = emit_conv2_unit(nc, pps, cw2T, m2, fm1, b_c2, mi, ch,
                                         pf_state["k"], pf_state["k"] + take,
                                         ps=pf_state["ps"])
                    pf_state["ps"] = ps
                    pf_state["k"] += take
                    nmm -= take
                    if pf_state["k"] == 27:
                        pf_state["u"] += 1
                        pf_state["k"] = 0
                        pf_state["ps"] = None

            def ydiag_ap(img, i0, j0, kw, c, B):
                """[kw, B] wavefront-diagonal AP into padded img tile chunk c."""
                off = (i0 + 2) * WP + (j0 + 2)
                return _ap(img, c * HP * WP, off, [[2 * HP * WP, kw], [DIAG, B]])

            def emit_seed2(pt, slot, brow_ap, mw, B):
                nc.tensor.matmul(pt[0:mw, slot, 0:B], brow_ap, ones[0:1, 0:B],
                                 start=True, stop=False, skip_group_check=True)

            def emit_old_ctx(t1):
                """ctx_b seed for step t1's ctx -> zc[t1%2][5:8]; return tap MM list."""
                s1 = t1 % 2
                i_lo, B, j_lo = step_geom(t1)
                ms_list = chunks_of(384)
                for m, (ms, mw) in enumerate(ms_list):
                    emit_seed2(zc[s1], 5 + m, ctxb[0:1, ms:ms + mw], mw, B)
                mms = []
                for ti, (dy, dx) in enumerate(OLD_TAPS):
                    for c, (cs, kw) in enumerate(chunks_of(192)):
                        for m, (ms, mw) in enumerate(ms_list):
                            mms.append((ti, dy, dx, c, cs, kw, m, ms, mw))
                return i_lo, B, j_lo, mms

            # prologue part of step 0's z0/ctx accumulation
            def emit_z0_pre(t1):
                """seeds + f-part + (later) ctx-part for z0 of step t1 -> zc[t1%2][0:5]"""
                s1 = t1 % 2
                i_lo, B, j_lo = step_geom(t1)
                for m, (ms, mw) in enumerate(chunks_of(640)):
                    emit_seed2(zc[s1], m, brow[0][0:1, ms:ms + mw], mw, B)
                for k in range(3):
                    for m, (ms, mw) in enumerate(chunks_of(640)):
                        rhs = _ap(fm1, k * NPIX, i_lo * W + j_lo,
                                  [[3 * NPIX, 128], [W - 3, B]])
                        nc.tensor.matmul(zc[s1][0:mw, m, 0:B],
                                         W0fT[0:128, k, ms:ms + mw], rhs,
                                         start=False, stop=False, skip_group_check=True)

            def emit_z0_ctx(t1):
                s1 = t1 % 2
                i_lo, B, j_lo = step_geom(t1)
                for k in range(3):
                    for m, (ms, mw) in enumerate(chunks_of(640)):
                        nc.tensor.matmul(zc[s1][0:mw, m, 0:B],
                                         W0cT[0:128, k, ms:ms + mw],
                                         Xc[0:128, k, 0:B],
                                         start=False, stop=False, skip_group_check=True)

            def emit_old_mms(t1, geom, mms):
                i_lo, B, j_lo = geom
                s1 = t1 % 2
                for (ti, dy, dx, c, cs, kw, m, ms, mw) in mms:
                    rhs = ydiag_ap(Yimg, i_lo - dy, j_lo - dx, kw, c, B)
                    last = (ti == len(OLD_TAPS) - 1) and (c == 1)
                    nc.tensor.matmul(zc[s1][0:mw, 5 + m, 0:B],
                                     WdT[OLD_TAPS[ti]][0:kw, c, ms:ms + mw], rhs,
                                     start=False, stop=last, skip_group_check=True)

            # --- step 0 pre-work (its sources are all zero borders) ---
            g0 = emit_old_ctx(0)
            emit_old_mms(0, (g0[0], g0[1], g0[2]), g0[3])
            i_lo0, B0, j_lo0 = step_geom(0)
            nc.vector.tensor_copy(Xc[:, 0:3, 0:B0], zc[0][:, 5:8, 0:B0])
            emit_z0_pre(0)
            emit_z0_ctx(0)

            KCHW = {li: chunks_of(DIMS[li]) for li in range(1, 6)}
            MCHW = {li: chunks_of(DIMS[li + 1]) for li in range(0, 6)}

            for t in range(nsteps):
                s = t % 2
                s1 = (t + 1) % 2
                i_lo, B, j_lo = step_geom(t)
                have_next = t + 1 < nsteps
                if have_next:
                    i_lo1, B1, j_lo1 = step_geom(t + 1)

                # ---- fresh taps -> z0 (critical) ----
                for m, (ms, mw) in enumerate(MCHW[0]):
                    for f in range(2):
                        dy, dx = FRESH_TAPS[f]
                        for c, (cs, kw) in enumerate(chunks_of(192)):
                            rhs = ydiag_ap(Yimg, i_lo - dy, j_lo - dx, kw, c, B)
                            nc.tensor.matmul(zc[s][0:mw, m, 0:B],
                                             GT[f][0:kw, c, ms:ms + mw], rhs,
                                             start=False,
                                             stop=(f == 1 and c == 1),
                                             skip_group_check=True)

                # ---- lrelu z0 -> x1 (critical ACT) ----
                nc.scalar.activation(xs[1][:, 0:5, 0:B], zc[s][:, 0:5, 0:B],
                                     mybir.ActivationFunctionType.Lrelu, alpha=0.01)

                # old-ctx for t+1 (fillers)
                old_mms = []
                if have_next:
                    g = emit_old_ctx(t + 1)
                    old_mms = g[3]
                    geom1 = (g[0], g[1], g[2])

                # ---- MLP layers 1..5 ----
                for li in range(1, 6):
                    # fillers before the critical MMs of this layer
                    if li == 2 and have_next:
                        emit_old_mms(t + 1, geom1, old_mms[:33])
                    elif li == 3 and have_next:
                        emit_old_mms(t + 1, geom1, old_mms[33:])
                    elif li == 4 and have_next:
                        nc.vector.tensor_copy(Xc[:, 0:3, 0:B1], zc[s1][:, 5:8, 0:B1])
                        emit_z0_pre(t + 1)
                        pfill(4)
                    elif li == 5 and have_next:
                        pfill(5)

                    # seed + main MMs -> zs
                    for m, (ms, mw) in enumerate(MCHW[li]):
                        emit_seed2(zs, ZOFF[li] + m, brow[li][0:1, ms:ms + mw], mw, B)
                    kch = KCHW[li]
                    for m, (ms, mw) in enumerate(MCHW[li]):
                        for k, (ks, kw) in enumerate(kch):
                            nc.tensor.matmul(zs[0:mw, ZOFF[li] + m, 0:B],
                                             WT[li][0:kw, k, ms:ms + mw],
                                             xs[li][0:kw, k, 0:B],
                                             start=False, stop=(k == len(kch) - 1),
                                             skip_group_check=True)
                    if li < 5:
                        nch = len(MCHW[li])
                        nc.scalar.activation(xs[li + 1][:, 0:nch, 0:B],
                                             zs[:, ZOFF[li]:ZOFF[li] + nch, 0:B],
                                             mybir.ActivationFunctionType.Lrelu,
                                             alpha=0.01)

                # ---- Y = z5 + w_hat (critical DVE) ----
                off = (i_lo + 2) * WP + (j_lo + 2)
                ydst = _ap(Yimg, 0, off, [[2 * HP * WP, 128], [HP * WP, 2], [DIAG, B]])
                ywim = _ap(wimg, 0, off, [[2 * HP * WP, 128], [HP * WP, 2], [DIAG, B]])
                nc.vector.tensor_tensor(ydst, zs[:, 12:14, 0:B], ywim,
                                        mybir.AluOpType.add)
                # late filler: ctx->z0 for t+1 (needs Xc evict from ~2 hops ago)
                if have_next:
                    emit_z0_ctx(t + 1)

            # ================= EPILOGUE =================
            with tc.tile_pool(name="epi", bufs=1) as epi:
                Yimg32 = epi.tile([128, 2, NPIX], F32)
                src = _ap(Yimg, 0, 2 * WP + 2,
                          [[2 * HP * WP, 128], [HP * WP, 2], [WP, H], [1, W]])
                dst = _ap(Yimg32, 0, 0,
                          [[2 * NPIX, 128], [NPIX, 2], [W, H], [1, W]])
                nc.vector.tensor_copy(dst, src)
                ov = out.ap()[0]
                for ci, (cs, cw) in enumerate(chunks_of(192)):
                    nc.sync.dma_start(
                        ov[cs:cs + cw],
                        Yimg32[0:cw, ci, :].rearrange("p (h w) -> p h w", h=H))

    nc.compile()
    return nc


def emit_conv2_unit(nc, pps, cw2T, m2, fm1, b_c2, mi, ch, k0, k1, ps=None):
    """Emit conv2 MMs [k0, k1) of unit (mi, ch); 27 MMs total per unit.
    MM index kk = k * 3 + si, k in 0..8 (3x3 tap), si in 0..2 (cin chunk)."""
    import concourse.mybir as mybir
    if ps is None:
        ps = pps.tile([128, 384], F32, tag="cps", name="cps")
    for kk in range(k0, k1):
        k, si = kk // 3, kk % 3
        ky, kx = k // 3, k % 3
        sw = 128 if si < 2 else 32
        lhsT = _ap(cw2T, (mi * 3 + si) * 9 * 128, k * 128,
                   [[3 * 3 * 9 * 128, 128], [1, 128]])
        rhs = _ap(m2, si * 34 * 50, (ky + 8 * ch) * 50 + kx,
                  [[3 * 34 * 50, 128], [50, 8], [1, 48]])
        nc.tensor.matmul(ps[:, 0:384], lhsT, rhs,
                         start=(kk == 0), stop=(kk == 26), skip_group_check=True)
    if k1 == 27:
        nc.scalar.activation(fm1[:, mi, ch * 384:(ch + 1) * 384], ps[:, 0:384],
                             mybir.ActivationFunctionType.Identity,
                             bias=b_c2[:, mi][:, None], alpha=0.0)
    return ps


_NC_CACHE = {}


def kernel(**inputs):
    from concourse.bass_utils import run_bass_kernel_spmd
    key = "full"
    if key not in _NC_CACHE:
        _NC_CACHE[key] = build()
    nc = _NC_CACHE[key]
    in_map = {k: np.ascontiguousarray(np.asarray(v, dtype=np.float32))
              for k, v in inputs.items()}
    res = run_bass_kernel_spmd(nc, [in_map] * 8, core_ids=list(range(8)))
    return res.results[0]['out']


if __name__ == "__main__":
    t = build(nsteps=int(sys.argv[1]) if len(sys.argv) > 1 else NSTEPS)
    print("build ok")
    from concourse.timeline_sim import TimelineSim
    est = TimelineSim(t).simulate()
    print(f"HW exec time: {est:.0f} ns")


# revision 17
# speedup vs baseline: 1.0050x; 1.0050x over previous
"""Trainium2 Bass kernel for nn_BEE_Bin2Symbol (hyper-decoder + masked-conv
autoregressive MLP decoder).

Architecture (v2, latency-oriented):
- Sequential phase runs all GEMMs in [C_out-on-partitions, B-pixels-free]
  orientation (weights stationary as lhsT, activations moving): matmul cost
  scales with B<=16, transposes disappear, and each layer's nonlinearity is a
  single DVE scalar_tensor_tensor op  max(0.01*z, z)  reading PSUM directly.
- 140 slope-3 anti-diagonal wavefronts. Per step the critical chain is
  fresh-tap MMs -> lrelu(z0) -> W1 MMs -> lrelu(z1) -> ... -> z5 -> Y-add.
  Everything else (old-tap ctx GEMMs, f-part, bias seeds, phase-P conv
  streaming) is issued into the PE gaps between the chain's engine hops.
- Fresh taps (age-1) use precomposed G = W0c @ Wd_tap so they feed z0
  directly; old taps accumulate a ctx PSUM a step ahead, evicted by the
  scalar engine (ACT) off the critical path.
- All sequential-phase weights/activations in bf16 (PSUM accumulate f32).
- Hyper-decoder (2 stride-2 deconvs + 3x3 conv) in f32r as phase-decomposed
  GEMMs; conv2's last 3 row-blocks stream into the first ~15 wavefront steps.
"""
import sys, os
sys.path.insert(0, "/opt/trn_rl_repo")

import numpy as np

import concourse.bass as bass
import concourse.bacc as bacc
import concourse.mybir as mybir
import concourse.tile as tile
from concourse.masks import make_identity

F32 = mybir.dt.float32
F32R = mybir.dt.float32r
BF16 = mybir.dt.bfloat16

H, W = 32, 48
HP, WP = H + 4, W + 4            # padded image 36 x 52
NPIX = H * W
NSTEPS = 3 * (H - 1) + (W - 1) + 1   # 140
DIAG = WP - 3                    # 49: wavefront-diagonal stride in padded img

# taps (dy, dx): source pixel = (i-dy, j-dx); ctx_w index (ky,kx) = (2-dy, 2-dx)
TAPS = [(2, 2), (2, 1), (2, 0), (2, -1), (2, -2),
        (1, 2), (1, 1), (1, 0), (1, -1), (1, -2),
        (0, 1), (0, 2)]
FRESH_TAPS = [(1, -2), (0, 1)]                # age-1 taps (need step t-1)
OLD_TAPS = [d for d in TAPS if d not in FRESH_TAPS]

DIMS = [768, 640, 512, 384, 320, 256, 192]   # MLP dims; layer l: DIMS[l]->DIMS[l+1]
# z1..z5 chunk offsets inside the shared zs psum tile [128, 14, 16]
ZOFF = {1: 0, 2: 4, 3: 7, 4: 10, 5: 12}


def cdiv(a, b):
    return (a + b - 1) // b


def chunks_of(n, c=128):
    return [(s, min(c, n - s)) for s in range(0, n, c)]


def _ap(tile_ap, slot_off, elem_off, plist):
    """Build a custom AP into a [128, S, F]-shaped sbuf/psum tile."""
    base = tile_ap[:]
    return bass.AP(base.tensor, base.offset + slot_off + elem_off, plist)


def step_geom(t):
    i_lo = max(0, cdiv(t - (W - 1), 3))
    i_hi = min(H - 1, t // 3)
    return i_lo, i_hi - i_lo + 1, t - 3 * i_lo


def build(nsteps=NSTEPS):
    nc = bacc.Bacc()

    # ---------------- DRAM I/O ----------------
    di = {}
    di['z_hat'] = nc.dram_tensor('z_hat', [1, 192, 8, 12], F32, kind="ExternalInput")
    di['w_hat'] = nc.dram_tensor('w_hat', [1, 192, 32, 48], F32, kind="ExternalInput")
    di['hs_dw0'] = nc.dram_tensor('hs_dw0', [192, 192, 5, 5], F32, kind="ExternalInput")
    di['hs_db0'] = nc.dram_tensor('hs_db0', [192], F32, kind="ExternalInput")
    di['hs_dw1'] = nc.dram_tensor('hs_dw1', [192, 288, 5, 5], F32, kind="ExternalInput")
    di['hs_db1'] = nc.dram_tensor('hs_db1', [288], F32, kind="ExternalInput")
    di['hs_cw2'] = nc.dram_tensor('hs_cw2', [384, 288, 3, 3], F32, kind="ExternalInput")
    di['hs_cb2'] = nc.dram_tensor('hs_cb2', [384], F32, kind="ExternalInput")
    di['ctx_w'] = nc.dram_tensor('ctx_w', [384, 192, 5, 5], F32, kind="ExternalInput")
    di['ctx_b'] = nc.dram_tensor('ctx_b', [384], F32, kind="ExternalInput")
    for li in range(6):
        di[f'ep_w{li}'] = nc.dram_tensor(f'ep_w{li}', [DIMS[li + 1], DIMS[li]], F32,
                                         kind="ExternalInput")
        di[f'ep_b{li}'] = nc.dram_tensor(f'ep_b{li}', [DIMS[li + 1]], F32,
                                         kind="ExternalInput")
    out = nc.dram_tensor('out', [1, 192, 32, 48], F32, kind="ExternalOutput")

    with tile.TileContext(nc) as tc:
        with tc.tile_pool(name="pp", bufs=1) as pp, \
             tc.tile_pool(name="pps", bufs=1, space="PSUM") as pps:

            ident = pp.tile([128, 128], F32)
            make_identity(nc, ident[:])

            # ---------- persistent state ----------
            Yimg = pp.tile([128, 2, HP * WP], BF16)   # decoded image (padded)
            nc.gpsimd.memset(Yimg[:], 0.0)
            wimg = pp.tile([128, 2, HP * WP], F32)    # w_hat residual (padded)
            nc.gpsimd.memset(wimg[:], 0.0)
            fm1 = pp.tile([128, 3, NPIX], BF16)       # conv2 output [384, 1536]
            m2 = pp.tile([128, 3, 34 * 50], BF16)     # deconv1 out (padded 34x50)
            nc.gpsimd.memset(m2[:], 0.0)
            cw2T = pp.tile([128, 3, 3, 9 * 128], BF16)  # [cin, mi, si, k*128+o]

            # transposed weights (bf16)
            W0fT = pp.tile([128, 3, 640], BF16)
            W0cT = pp.tile([128, 3, 640], BF16)
            WT = {}
            for li in range(1, 6):
                WT[li] = pp.tile([128, cdiv(DIMS[li], 128), DIMS[li + 1]], BF16,
                                 tag=f"W{li}T", name=f"W{li}T")
            GT = [pp.tile([128, 2, 640], BF16, tag=f"GT{k}", name=f"GT{k}") for k in range(2)]
            WdT = {}
            for d in OLD_TAPS:
                WdT[d] = pp.tile([128, 2, 384], BF16, tag=f"Wd{d[0]}_{d[1]}", name=f"Wd{d[0]}_{d[1]}")

            # bias rows (lhsT for K=1 seed matmuls)
            brow = [pp.tile([1, DIMS[li + 1]], BF16, tag=f"b{li}", name=f"b{li}") for li in range(6)]
            ctxb = pp.tile([1, 384], BF16)
            ones = pp.tile([1, 16], BF16)
            nc.vector.memset(ones[:], 1.0)

            # sequential-phase activations (single tiles; WAR handled by sems)
            xs = {li: pp.tile([128, cdiv(DIMS[li], 128), 16], BF16, tag=f"x{li}", name=f"x{li}")
                  for li in range(1, 6)}
            Xc = pp.tile([128, 3, 16], BF16)          # evicted old-ctx

            # persistent psum: z0+ctx ring (2 banks), mlp zs (1), conv stream (1)
            zc = [pps.tile([128, 8, 16], F32, tag=f"zc{s}", name=f"zc{s}") for s in range(2)]
            zs = pps.tile([128, 14, 16], F32, tag="zs")
            nc.vector.memset(zs[:], 0.0)
            nc.vector.memset(zc[0][:], 0.0)
            nc.vector.memset(zc[1][:], 0.0)

            # ================= PROLOGUE =================
            with tc.tile_pool(name="pro", bufs=2) as pro, \
                 tc.tile_pool(name="prps", bufs=2, space="PSUM") as prps:

                # --- w_hat into padded residual image ---
                whv = di['w_hat'].ap()[0]
                for ci, (s, cw) in enumerate(chunks_of(192)):
                    dst = _ap(wimg, ci * HP * WP, 2 * WP + 2,
                              [[2 * HP * WP, cw], [WP, H], [1, W]])
                    nc.sync.dma_start(dst, whv[s:s + cw])

                # --- bias rows ---
                def load_brow(dst, dram, n):
                    st = pro.tile([1, 640], F32, tag="brs", name="brs")
                    nc.sync.dma_start(st[0:1, 0:n], dram.ap()[None, :])
                    nc.vector.tensor_copy(dst[0:1, 0:n], st[0:1, 0:n])
                for li in range(6):
                    load_brow(brow[li], di[f'ep_b{li}'], DIMS[li + 1])
                load_brow(ctxb, di['ctx_b'], 384)

                # --- transpose ep_w0..5 -> bf16 (alternate DVE/ACT for evicts) ---
                W0cT32 = pro.tile([128, 3, 640], F32R, tag="w0c32", bufs=1)
                tp_count = [0]
                def evict(dst_ap, src_ap):
                    if tp_count[0] % 2 == 0:
                        nc.vector.tensor_copy(dst_ap, src_ap)
                    else:
                        nc.scalar.activation(dst_ap, src_ap,
                                             mybir.ActivationFunctionType.Copy)
                    tp_count[0] += 1

                def load_and_transpose(li):
                    n_out, n_in = DIMS[li + 1], DIMS[li]
                    wnat = pro.tile([128, 6, 768], F32, tag="wnat", bufs=1)
                    for mi, (ms, mw) in enumerate(chunks_of(n_out)):
                        nc.sync.dma_start(wnat[0:mw, mi, 0:n_in],
                                          di[f'ep_w{li}'].ap()[ms:ms + mw])
                    for ci, (cs, cww) in enumerate(chunks_of(n_in)):
                        for mi, (ms, mw) in enumerate(chunks_of(n_out)):
                            pt = prps.tile([128, 128], F32, tag="tp")
                            nc.tensor.transpose(pt[0:cww, 0:mw],
                                                wnat[0:mw, mi, cs:cs + cww],
                                                ident[0:mw, 0:mw])
                            if li == 0:
                                if ci < 3:
                                    evict(W0fT[0:cww, ci, ms:ms + mw], pt[0:cww, 0:mw])
                                else:
                                    # ctx half: need bf16 AND f32 (for G compose)
                                    nc.vector.tensor_copy(
                                        W0cT[0:cww, ci - 3, ms:ms + mw], pt[0:cww, 0:mw])
                                    nc.vector.tensor_copy(
                                        W0cT32[0:cww, ci - 3, ms:ms + mw], pt[0:cww, 0:mw])
                            else:
                                evict(WT[li][0:cww, ci, ms:ms + mw], pt[0:cww, 0:mw])
                for li in range(6):
                    load_and_transpose(li)

                # --- ctx_w rows; WdT (old taps) via transpose; GT (fresh) compose ---
                cwn = [pro.tile([128, 192 * 25], F32R, tag=f"cwn{mi}", bufs=1, name=f"cwn{mi}")
                       for mi in range(3)]
                for mi in range(3):
                    nc.sync.dma_start(
                        cwn[mi][:],
                        di['ctx_w'].ap()[mi * 128:(mi + 1) * 128]
                        .rearrange("o c kh kw -> o (c kh kw)").bitcast(F32R))
                for d in OLD_TAPS:
                    ky, kx = 2 - d[0], 2 - d[1]
                    for mi in range(3):
                        for ci, (cs, cww) in enumerate(chunks_of(192)):
                            src = _ap(cwn[mi], 0, cs * 25 + ky * 5 + kx,
                                      [[192 * 25, 128], [25, cww]]).bitcast(F32)
                            pt = prps.tile([128, 128], F32, tag="tp")
                            nc.tensor.transpose(pt[0:cww, 0:128], src, ident[:])
                            evict(WdT[d][0:cww, ci, mi * 128:(mi + 1) * 128],
                                  pt[0:cww, 0:128])
                # GT[f] = (W0c @ Wd_tap)^T = Wd^T W0c^T : out [192, 640]
                for f, d in enumerate(FRESH_TAPS):
                    ky, kx = 2 - d[0], 2 - d[1]
                    for mc, (cs, cww) in enumerate(chunks_of(192)):
                        for nh in range(2):   # 640 -> 2 x 320
                            gp = prps.tile([128, 384], F32, tag="dps")
                            for ki in range(3):
                                lhsT = _ap(cwn[ki], 0, cs * 25 + ky * 5 + kx,
                                           [[192 * 25, 128], [25, cww]])
                                rhs = W0cT32[0:128, ki, nh * 320:(nh + 1) * 320]
                                nc.tensor.matmul(gp[0:cww, 0:320], lhsT, rhs,
                                                 start=(ki == 0), stop=(ki == 2),
                                                 skip_group_check=True)
                            evict(GT[f][0:cww, mc, nh * 320:(nh + 1) * 320],
                                  gp[0:cww, 0:320])

            # ============ PROLOGUE B: hyper-decoder ============
            with tc.tile_pool(name="proB", bufs=2) as pro, \
                 tc.tile_pool(name="prpsB", bufs=2, space="PSUM") as prps:

                # --- deconv stack (f32r, phase-decomposed) ---
                def load_bias_col(name, n):
                    nch = cdiv(n, 128)
                    t = pp.tile([128, nch], F32, tag=f"b_{name}", name=f"b_{name}")
                    nc.vector.memset(t[:], 0.0)
                    for ci, (s, w_) in enumerate(chunks_of(n)):
                        nc.sync.dma_start(t[0:w_, ci:ci + 1], di[name][s:s + w_][:, None])
                    return t
                b_d0 = load_bias_col('hs_db0', 192)
                b_d1 = load_bias_col('hs_db1', 288)
                b_c2 = load_bias_col('hs_cb2', 384)

                def deconv_chunk(inp_t, inp_hw, w_t, cin, mw, mi, out_t, bias_t):
                    hi, wi = inp_hw
                    ip_w = wi + 2
                    op_w = 2 * wi + 2
                    for py in range(2):
                        for px in range(2):
                            ps = prps.tile([128, 16 * 24], F32, tag="dps")
                            first = True
                            taps = [(u, v) for u in range(py, 5, 2) for v in range(px, 5, 2)]
                            for ti, (u, v) in enumerate(taps):
                                dy = (py + 2 - u) // 2
                                dx = (px + 2 - v) // 2
                                for ci in range(cdiv(cin, 128)):
                                    lhsT = _ap(w_t, ci * 128 * 25, u * 5 + v,
                                               [[2 * 128 * 25, 128], [25, mw]])
                                    rhs = _ap(inp_t, ci * (hi + 2) * ip_w,
                                              (1 + dy) * ip_w + (1 + dx),
                                              [[2 * (hi + 2) * ip_w, 128], [ip_w, hi], [1, wi]])
                                    last = (ti == len(taps) - 1) and (ci == cdiv(cin, 128) - 1)
                                    nc.tensor.matmul(ps[0:mw, 0:hi * wi], lhsT, rhs,
                                                     start=first, stop=last,
                                                     skip_group_check=True)
                                    first = False
                            dst = _ap(out_t, mi * (2 * hi + 2) * op_w,
                                      (py + 1) * op_w + (px + 1),
                                      [[out_t.shape[1] * (2 * hi + 2) * op_w, mw],
                                       [2 * op_w, hi], [2, wi]])
                            nc.scalar.activation(
                                dst, ps[0:mw, 0:hi * wi].rearrange("p (a b) -> p a b", a=hi),
                                mybir.ActivationFunctionType.Lrelu,
                                bias=bias_t[0:mw, mi][:, None], alpha=0.01)

                # deconv0: z[192,8,12] -> m1[192,16,24]
                m1 = pro.tile([128, 2, 18 * 26], F32R, tag="m1", bufs=1)
                nc.gpsimd.memset(m1[:].bitcast(F32), 0.0)
                zp = pro.tile([128, 2, 10 * 14], F32R, tag="zp", bufs=1)
                nc.gpsimd.memset(zp[:].bitcast(F32), 0.0)
                zv = di['z_hat'].ap()[0]
                for ci, (s, cw) in enumerate(chunks_of(192)):
                    dst = _ap(zp, ci * 140, 14 + 1, [[2 * 140, cw], [14, 8], [1, 12]])
                    nc.sync.dma_start(dst, zv[s:s + cw].bitcast(F32R))
                for mi, (ms, mw) in enumerate(chunks_of(192)):
                    dw = pro.tile([128, 2, 128 * 25], F32R, tag="dw")
                    nc.gpsimd.memset(dw[64:128, 1, :].bitcast(F32), 0.0)
                    for ci, (cs, cww) in enumerate(chunks_of(192)):
                        nc.sync.dma_start(
                            dw[0:cww, ci, 0:mw * 25],
                            di['hs_dw0'].ap()[cs:cs + cww, ms:ms + mw]
                            .rearrange("c o kh kw -> c (o kh kw)").bitcast(F32R))
                    deconv_chunk(zp, (8, 12), dw, 192, mw, mi, m1, b_d0)

                # deconv1: m1[192,16,24] -> m2[288,32,48]
                for mi, (ms, mw) in enumerate(chunks_of(288)):
                    dw = pro.tile([128, 2, 128 * 25], F32R, tag="dw")
                    nc.gpsimd.memset(dw[64:128, 1, :].bitcast(F32), 0.0)
                    for ci, (cs, cww) in enumerate(chunks_of(192)):
                        nc.sync.dma_start(
                            dw[0:cww, ci, 0:mw * 25],
                            di['hs_dw1'].ap()[cs:cs + cww, ms:ms + mw]
                            .rearrange("c o kh kw -> c (o kh kw)").bitcast(F32R))
                    deconv_chunk(m1, (16, 24), dw, 192, mw, mi, m2, b_d1)

                # conv2 weights: cw2T[cin, mi, si, k*128+o] via PE transpose
                for mi in range(3):
                    cw2s = pro.tile([128, 288 * 9], F32, tag="cw2s")
                    nc.sync.dma_start(
                        cw2s[:],
                        di['hs_cw2'].ap()[mi * 128:(mi + 1) * 128]
                        .rearrange("o c kh kw -> o (c kh kw)"))
                    nc.vector.memset(cw2T[32:64, mi, 2, :], 0.0)
                    nc.gpsimd.memset(cw2T[64:128, mi, 2, :], 0.0)
                    for k in range(9):
                        for si, (ss, sw) in enumerate(chunks_of(288)):
                            src = _ap(cw2s, 0, ss * 9 + k, [[288 * 9, 128], [9, sw]])
                            pt = prps.tile([128, 128], F32, tag="tp")
                            nc.tensor.transpose(pt[0:sw, 0:128], src, ident[:])
                            evict(_ap(cw2T, (mi * 3 + si) * 9 * 128, k * 128,
                                      [[3 * 3 * 9 * 128, sw], [1, 128]]),
                                  pt[0:sw, 0:128])

                # conv2 row-block 0 (rows 0..7) upfront
                for mi in range(3):
                    emit_conv2_unit(nc, pps, cw2T, m2, fm1, b_c2, mi, 0, 0, 27)

            # ================= SEQUENTIAL PHASE =================
            # conv2 streaming state: remaining units (mi, ch>=1), 27 MMs each
            pf_units = [(mi, ch) for ch in range(1, 4) for mi in range(3)]
            pf_state = {"u": 0, "k": 0, "ps": None}

            def pfill(nmm):
                while nmm > 0 and pf_state["u"] < len(pf_units):
                    mi, ch = pf_units[pf_state["u"]]
                    take = min(nmm, 27 - pf_state["k"])
                    ps = emit_conv2_unit(nc, pps, cw2T, m2, fm1, b_c2, mi, ch,
                                         pf_state["k"], pf_state["k"] + take,
                                         ps=pf_state["ps"])
                    pf_state["ps"] = ps
                    pf_state["k"] += take
                    nmm -= take
                    if pf_state["k"] == 27:
                        pf_state["u"] += 1
                        pf_state["k"] = 0
                        pf_state["ps"] = None

            def ydiag_ap(img, i0, j0, kw, c, B):
                """[kw, B] wavefront-diagonal AP into padded img tile chunk c."""
                off = (i0 + 2) * WP + (j0 + 2)
                return _ap(img, c * HP * WP, off, [[2 * HP * WP, kw], [DIAG, B]])

            def emit_seed2(pt, slot, brow_ap, mw, B):
                nc.tensor.matmul(pt[0:mw, slot, 0:B], brow_ap, ones[0:1, 0:B],
                                 start=True, stop=False, skip_group_check=True)

            def emit_old_ctx(t1):
                """ctx_b seed for step t1's ctx -> zc[t1%2][5:8]; return tap MM list."""
                s1 = t1 % 2
                i_lo, B, j_lo = step_geom(t1)
                ms_list = chunks_of(384)
                for m, (ms, mw) in enumerate(ms_list):
                    emit_seed2(zc[s1], 5 + m, ctxb[0:1, ms:ms + mw], mw, B)
                mms = []
                for ti, (dy, dx) in enumerate(OLD_TAPS):
                    for c, (cs, kw) in enumerate(chunks_of(192)):
                        for m, (ms, mw) in enumerate(ms_list):
                            mms.append((ti, dy, dx, c, cs, kw, m, ms, mw))
                return i_lo, B, j_lo, mms

            # prologue part of step 0's z0/ctx accumulation
            def emit_z0_pre(t1):
                """seeds + f-part + (later) ctx-part for z0 of step t1 -> zc[t1%2][0:5]"""
                s1 = t1 % 2
                i_lo, B, j_lo = step_geom(t1)
                for m, (ms, mw) in enumerate(chunks_of(640)):
                    emit_seed2(zc[s1], m, brow[0][0:1, ms:ms + mw], mw, B)
                for k in range(3):
                    for m, (ms, mw) in enumerate(chunks_of(640)):
                        rhs = _ap(fm1, k * NPIX, i_lo * W + j_lo,
                                  [[3 * NPIX, 128], [W - 3, B]])
                        nc.tensor.matmul(zc[s1][0:mw, m, 0:B],
                                         W0fT[0:128, k, ms:ms + mw], rhs,
                                         start=False, stop=False, skip_group_check=True)

            def emit_z0_ctx(t1):
                s1 = t1 % 2
                i_lo, B, j_lo = step_geom(t1)
                for k in range(3):
                    for m, (ms, mw) in enumerate(chunks_of(640)):
                        nc.tensor.matmul(zc[s1][0:mw, m, 0:B],
                                         W0cT[0:128, k, ms:ms + mw],
                                         Xc[0:128, k, 0:B],
                                         start=False, stop=False, skip_group_check=True)

            def emit_old_mms(t1, geom, mms):
                i_lo, B, j_lo = geom
                s1 = t1 % 2
                for (ti, dy, dx, c, cs, kw, m, ms, mw) in mms:
                    rhs = ydiag_ap(Yimg, i_lo - dy, j_lo - dx, kw, c, B)
                    last = (ti == len(OLD_TAPS) - 1) and (c == 1)
                    nc.tensor.matmul(zc[s1][0:mw, 5 + m, 0:B],
                                     WdT[OLD_TAPS[ti]][0:kw, c, ms:ms + mw], rhs,
                                     start=False, stop=last, skip_group_check=True)

            # --- step 0 pre-work (its sources are all zero borders) ---
            g0 = emit_old_ctx(0)
            emit_old_mms(0, (g0[0], g0[1], g0[2]), g0[3])
            i_lo0, B0, j_lo0 = step_geom(0)
            nc.vector.tensor_copy(Xc[:, 0:3, 0:B0], zc[0][:, 5:8, 0:B0])
            emit_z0_pre(0)
            emit_z0_ctx(0)

            KCHW = {li: chunks_of(DIMS[li]) for li in range(1, 6)}
            MCHW = {li: chunks_of(DIMS[li + 1]) for li in range(0, 6)}

            for t in range(nsteps):
                s = t % 2
                s1 = (t + 1) % 2
                i_lo, B, j_lo = step_geom(t)
                have_next = t + 1 < nsteps
                if have_next:
                    i_lo1, B1, j_lo1 = step_geom(t + 1)

                # ---- fresh taps -> z0 (critical) ----
                for m, (ms, mw) in enumerate(MCHW[0]):
                    for f in range(2):
                        dy, dx = FRESH_TAPS[f]
                        for c, (cs, kw) in enumerate(chunks_of(192)):
                            rhs = ydiag_ap(Yimg, i_lo - dy, j_lo - dx, kw, c, B)
                            nc.tensor.matmul(zc[s][0:mw, m, 0:B],
                                             GT[f][0:kw, c, ms:ms + mw], rhs,
                                             start=False,
                                             stop=(f == 1 and c == 1),
                                             skip_group_check=True)

                # ---- lrelu z0 -> x1 (critical ACT) ----
                nc.scalar.activation(xs[1][:, 0:5, 0:B], zc[s][:, 0:5, 0:B],
                                     mybir.ActivationFunctionType.Lrelu, alpha=0.01)

                # old-ctx for t+1 (fillers)
                old_mms = []
                if have_next:
                    g = emit_old_ctx(t + 1)
                    old_mms = g[3]
                    geom1 = (g[0], g[1], g[2])

                # ---- MLP layers 1..5 ----
                for li in range(1, 6):
                    # fillers before the critical MMs of this layer
                    if li == 2 and have_next:
                        emit_old_mms(t + 1, geom1, old_mms[:33])
                    elif li == 3 and have_next:
                        emit_old_mms(t + 1, geom1, old_mms[33:])
                    elif li == 4 and have_next:
                        nc.vector.tensor_copy(Xc[:, 0:3, 0:B1], zc[s1][:, 5:8, 0:B1])
                        emit_z0_pre(t + 1)
                        pfill(4)
                    elif li == 5 and have_next:
                        pfill(5)

                    # seed + main MMs -> zs
                    for m, (ms, mw) in enumerate(MCHW[li]):
                        emit_seed2(zs, ZOFF[li] + m, brow[li][0:1, ms:ms + mw], mw, B)
                    kch = KCHW[li]
                    for m, (ms, mw) in enumerate(MCHW[li]):
                        for k, (ks, kw) in enumerate(kch):
                            nc.tensor.matmul(zs[0:mw, ZOFF[li] + m, 0:B],
                                             WT[li][0:kw, k, ms:ms + mw],
                                             xs[li][0:kw, k, 0:B],
                                             start=False, stop=(k == len(kch) - 1),
                                             skip_group_check=True)
                    if li < 5:
                        nch = len(MCHW[li])
                        nc.scalar.activation(xs[li + 1][:, 0:nch, 0:B],
                                             zs[:, ZOFF[li]:ZOFF[li] + nch, 0:B],
                                             mybir.ActivationFunctionType.Lrelu,
                                             alpha=0.01)

                # ---- Y = z5 + w_hat (critical DVE) ----
                off = (i_lo + 2) * WP + (j_lo + 2)
                ydst = _ap(Yimg, 0, off, [[2 * HP * WP, 128], [HP * WP, 2], [DIAG, B]])
                ywim = _ap(wimg, 0, off, [[2 * HP * WP, 128], [HP * WP, 2], [DIAG, B]])
                nc.vector.tensor_tensor(ydst, zs[:, 12:14, 0:B], ywim,
                                        mybir.AluOpType.add)
                # late filler: ctx->z0 for t+1 (needs Xc evict from ~2 hops ago)
                if have_next:
                    emit_z0_ctx(t + 1)

            # ================= EPILOGUE =================
            with tc.tile_pool(name="epi", bufs=1) as epi:
                Yimg32 = epi.tile([128, 2, NPIX], F32)
                src = _ap(Yimg, 0, 2 * WP + 2,
                          [[2 * HP * WP, 128], [HP * WP, 2], [WP, H], [1, W]])
                dst = _ap(Yimg32, 0, 0,
                          [[2 * NPIX, 128], [NPIX, 2], [W, H], [1, W]])
                nc.vector.tensor_copy(dst, src)
                ov = out.ap()[0]
                for ci, (cs, cw) in enumerate(chunks_of(192)):
                    nc.sync.dma_start(
                        ov[cs:cs + cw],
                        Yimg32[0:cw, ci, :].rearrange("p (h w) -> p h w", h=H))

    nc.compile()
    return nc


def emit_conv2_unit(nc, pps, cw2T, m2, fm1, b_c2, mi, ch, k0, k1, ps=None):
    """Emit conv2 MMs [k0, k1) of unit (mi, ch); 27 MMs total per unit.
    MM index kk = k * 3 + si, k in 0..8 (3x3 tap), si in 0..2 (cin chunk)."""
    import concourse.mybir as mybir
    if ps is None:
        ps = pps.tile([128, 384], F32, tag="cps", name="cps")
    for kk in range(k0, k1):
        k, si = kk // 3, kk % 3
        ky, kx = k // 3, k % 3
        sw = 128 if si < 2 else 32
        lhsT = _ap(cw2T, (mi * 3 + si) * 9 * 128, k * 128,
                   [[3 * 3 * 9 * 128, 128], [1, 128]])
        rhs = _ap(m2, si * 34 * 50, (ky + 8 * ch) * 50 + kx,
                  [[3 * 34 * 50, 128], [50, 8], [1, 48]])
        nc.tensor.matmul(ps[:, 0:384], lhsT, rhs,
                         start=(kk == 0), stop=(kk == 26), skip_group_check=True)
    if k1 == 27:
        nc.scalar.activation(fm1[:, mi, ch * 384:(ch + 1) * 384], ps[:, 0:384],
                             mybir.ActivationFunctionType.Identity,
                             bias=b_c2[:, mi][:, None], alpha=0.0)
    return ps


_NC_CACHE = {}


def kernel(**inputs):
    from concourse.bass_utils import run_bass_kernel_spmd
    key = "full"
    if key not in _NC_CACHE:
        _NC_CACHE[key] = build()
    nc = _NC_CACHE[key]
    in_map = {k: np.ascontiguousarray(np.asarray(v, dtype=np.float32))
              for k, v in inputs.items()}
    res = run_bass_kernel_spmd(nc, [in_map] * 8, core_ids=list(range(8)))
    return res.results[0]['out']


if __name__ == "__main__":
    t = build(nsteps=int(sys.argv[1]) if len(sys.argv) > 1 else NSTEPS)
    print("build ok")
    from concourse.timeline_sim import TimelineSim
    est = TimelineSim(t).simulate()
    print(f"HW exec time: {est:.0f} ns")


# revision 20
# speedup vs baseline: 2.5328x; 2.5202x over previous
"""Trainium2 Bass kernel for nn_BEE_Bin2Symbol (hyper-decoder + masked-conv
autoregressive MLP decoder).

Architecture (v2, latency-oriented):
- Sequential phase runs all GEMMs in [C_out-on-partitions, B-pixels-free]
  orientation (weights stationary as lhsT, activations moving): matmul cost
  scales with B<=16, transposes disappear, and each layer's nonlinearity is a
  single DVE scalar_tensor_tensor op  max(0.01*z, z)  reading PSUM directly.
- 140 slope-3 anti-diagonal wavefronts. Per step the critical chain is
  fresh-tap MMs -> lrelu(z0) -> W1 MMs -> lrelu(z1) -> ... -> z5 -> Y-add.
  Everything else (old-tap ctx GEMMs, f-part, bias seeds, phase-P conv
  streaming) is issued into the PE gaps between the chain's engine hops.
- Fresh taps (age-1) use precomposed G = W0c @ Wd_tap so they feed z0
  directly; old taps accumulate a ctx PSUM a step ahead, evicted by the
  scalar engine (ACT) off the critical path.
- All sequential-phase weights/activations in bf16 (PSUM accumulate f32).
- Hyper-decoder (2 stride-2 deconvs + 3x3 conv) in f32r as phase-decomposed
  GEMMs; conv2's last 3 row-blocks stream into the first ~15 wavefront steps.
"""
import sys, os
sys.path.insert(0, "/opt/trn_rl_repo")

import numpy as np

import concourse.bass as bass
import concourse.bacc as bacc
import concourse.mybir as mybir
import concourse.tile as tile
from concourse.masks import make_identity

F32 = mybir.dt.float32
F32R = mybir.dt.float32r
BF16 = mybir.dt.bfloat16

H, W = 32, 48
HP, WP = H + 4, W + 4            # padded image 36 x 52
NPIX = H * W
NSTEPS = 3 * (H - 1) + (W - 1) + 1   # 140
DIAG = WP - 3                    # 49: wavefront-diagonal stride in padded img

# taps (dy, dx): source pixel = (i-dy, j-dx); ctx_w index (ky,kx) = (2-dy, 2-dx)
TAPS = [(2, 2), (2, 1), (2, 0), (2, -1), (2, -2),
        (1, 2), (1, 1), (1, 0), (1, -1), (1, -2),
        (0, 1), (0, 2)]
FRESH_TAPS = [(1, -2), (0, 1)]                # age-1 taps (need step t-1)
OLD_TAPS = [d for d in TAPS if d not in FRESH_TAPS]

DIMS = [768, 640, 512, 384, 320, 256, 192]   # MLP dims; layer l: DIMS[l]->DIMS[l+1]
# z1..z5 chunk offsets inside the shared zs psum tile [128, 14, 16]
ZOFF = {1: 0, 2: 4, 3: 7, 4: 10, 5: 12}


def cdiv(a, b):
    return (a + b - 1) // b


def chunks_of(n, c=128):
    return [(s, min(c, n - s)) for s in range(0, n, c)]


def _ap(tile_ap, slot_off, elem_off, plist):
    """Build a custom AP into a [128, S, F]-shaped sbuf/psum tile."""
    base = tile_ap[:]
    return bass.AP(base.tensor, base.offset + slot_off + elem_off, plist)


def step_geom(t):
    i_lo = max(0, cdiv(t - (W - 1), 3))
    i_hi = min(H - 1, t // 3)
    return i_lo, i_hi - i_lo + 1, t - 3 * i_lo


def build(nsteps=NSTEPS):
    nc = bacc.Bacc()

    # ---------------- DRAM I/O ----------------
    di = {}
    di['z_hat'] = nc.dram_tensor('z_hat', [1, 192, 8, 12], F32, kind="ExternalInput")
    di['w_hat'] = nc.dram_tensor('w_hat', [1, 192, 32, 48], F32, kind="ExternalInput")
    di['hs_dw0'] = nc.dram_tensor('hs_dw0', [192, 192, 5, 5], F32, kind="ExternalInput")
    di['hs_db0'] = nc.dram_tensor('hs_db0', [192], F32, kind="ExternalInput")
    di['hs_dw1'] = nc.dram_tensor('hs_dw1', [192, 288, 5, 5], F32, kind="ExternalInput")
    di['hs_db1'] = nc.dram_tensor('hs_db1', [288], F32, kind="ExternalInput")
    di['hs_cw2'] = nc.dram_tensor('hs_cw2', [384, 288, 3, 3], F32, kind="ExternalInput")
    di['hs_cb2'] = nc.dram_tensor('hs_cb2', [384], F32, kind="ExternalInput")
    di['ctx_w'] = nc.dram_tensor('ctx_w', [384, 192, 5, 5], F32, kind="ExternalInput")
    di['ctx_b'] = nc.dram_tensor('ctx_b', [384], F32, kind="ExternalInput")
    for li in range(6):
        di[f'ep_w{li}'] = nc.dram_tensor(f'ep_w{li}', [DIMS[li + 1], DIMS[li]], F32,
                                         kind="ExternalInput")
        di[f'ep_b{li}'] = nc.dram_tensor(f'ep_b{li}', [DIMS[li + 1]], F32,
                                         kind="ExternalInput")
    out = nc.dram_tensor('out', [1, 192, 32, 48], F32, kind="ExternalOutput")

    with tile.TileContext(nc) as tc:
        with tc.tile_pool(name="pp", bufs=1) as pp, \
             tc.tile_pool(name="pps", bufs=1, space="PSUM") as pps:

            ident = pp.tile([128, 128], F32)
            make_identity(nc, ident[:])

            # ---------- persistent state ----------
            Yimg = pp.tile([128, 2, HP * WP], BF16)   # decoded image (padded)
            nc.gpsimd.memset(Yimg[:], 0.0)
            wimg = pp.tile([128, 2, HP * WP], F32)    # w_hat residual (padded)
            nc.gpsimd.memset(wimg[:], 0.0)
            fm1 = pp.tile([128, 3, NPIX], BF16)       # conv2 output [384, 1536]
            m2 = pp.tile([128, 3, 34 * 50], BF16)     # deconv1 out (padded 34x50)
            nc.gpsimd.memset(m2[:], 0.0)
            cw2T = pp.tile([128, 3, 3, 9 * 128], BF16)  # [cin, mi, si, k*128+o]

            # transposed weights (bf16)
            W0fT = pp.tile([128, 3, 640], BF16)
            W0cT = pp.tile([128, 3, 640], BF16)
            WT = {}
            for li in range(1, 6):
                WT[li] = pp.tile([128, cdiv(DIMS[li], 128), DIMS[li + 1]], BF16,
                                 tag=f"W{li}T", name=f"W{li}T")
            GT = [pp.tile([128, 2, 640], BF16, tag=f"GT{k}", name=f"GT{k}") for k in range(2)]
            WdT = {}
            for d in OLD_TAPS:
                WdT[d] = pp.tile([128, 2, 384], BF16, tag=f"Wd{d[0]}_{d[1]}", name=f"Wd{d[0]}_{d[1]}")

            # bias rows (lhsT for K=1 seed matmuls)
            brow = [pp.tile([1, DIMS[li + 1]], BF16, tag=f"b{li}", name=f"b{li}") for li in range(6)]
            ctxb = pp.tile([1, 384], BF16)
            ones = pp.tile([1, 16], BF16)
            nc.vector.memset(ones[:], 1.0)

            # sequential-phase activations (single tiles; WAR handled by sems)
            xs = {li: pp.tile([128, cdiv(DIMS[li], 128), 16], BF16, tag=f"x{li}", name=f"x{li}")
                  for li in range(1, 6)}
            Xc = pp.tile([128, 3, 16], BF16)          # evicted old-ctx

            # persistent psum: z0+ctx ring (2 banks), mlp zs (1), conv stream (1)
            zc = [pps.tile([128, 8, 16], F32, tag=f"zc{s}", name=f"zc{s}") for s in range(2)]
            zs = pps.tile([128, 14, 16], F32, tag="zs")
            nc.vector.memset(zs[:], 0.0)
            nc.vector.memset(zc[0][:], 0.0)
            nc.vector.memset(zc[1][:], 0.0)

            # ============ PROLOGUE 1: hyper-decoder (DMA priority) ============
            with tc.tile_pool(name="proB", bufs=2) as pro, \
                 tc.tile_pool(name="prpsB", bufs=2, space="PSUM") as prps:

                # small DMAs first
                whv = di['w_hat'].ap()[0]
                for ci, (s, cw) in enumerate(chunks_of(192)):
                    dst = _ap(wimg, ci * HP * WP, 2 * WP + 2,
                              [[2 * HP * WP, cw], [WP, H], [1, W]])
                    nc.sync.dma_start(dst, whv[s:s + cw])

                def load_bias_col(name, n):
                    nch = cdiv(n, 128)
                    t = pp.tile([128, nch], F32, tag=f"b_{name}", name=f"b_{name}")
                    nc.vector.memset(t[:], 0.0)
                    for ci, (s, w_) in enumerate(chunks_of(n)):
                        nc.sync.dma_start(t[0:w_, ci:ci + 1], di[name][s:s + w_][:, None])
                    return t
                b_d0 = load_bias_col('hs_db0', 192)
                b_d1 = load_bias_col('hs_db1', 288)
                b_c2 = load_bias_col('hs_cb2', 384)

                def load_brow(dst, dram, n):
                    st = pro.tile([1, 640], F32, tag="brs", name="brs", bufs=3)
                    nc.sync.dma_start(st[0:1, 0:n], dram.ap()[None, :])
                    nc.vector.tensor_copy(dst[0:1, 0:n], st[0:1, 0:n])
                for li in range(6):
                    load_brow(brow[li], di[f'ep_b{li}'], DIMS[li + 1])
                load_brow(ctxb, di['ctx_b'], 384)

                # deconv input + weights (DMAs lead the queue)
                zp = pro.tile([128, 2, 10 * 14], F32R, tag="zp", bufs=1)
                nc.gpsimd.memset(zp[:].bitcast(F32), 0.0)
                zv = di['z_hat'].ap()[0]
                for ci, (s, cw) in enumerate(chunks_of(192)):
                    dst = _ap(zp, ci * 140, 14 + 1, [[2 * 140, cw], [14, 8], [1, 12]])
                    nc.sync.dma_start(dst, zv[s:s + cw].bitcast(F32R))

                m1 = pro.tile([128, 2, 18 * 26], F32R, tag="m1", bufs=1)
                nc.gpsimd.memset(m1[:].bitcast(F32), 0.0)

                dw0t, dw1t = [], []
                for mi, (ms, mw) in enumerate(chunks_of(192)):
                    dw = pro.tile([128, 2, 128 * 25], F32R, tag="dw", name="dw", bufs=3)
                    nc.gpsimd.memset(dw[64:128, 1, :].bitcast(F32), 0.0)
                    for ci, (cs, cww) in enumerate(chunks_of(192)):
                        nc.sync.dma_start(
                            dw[0:cww, ci, 0:mw * 25],
                            di['hs_dw0'].ap()[cs:cs + cww, ms:ms + mw]
                            .rearrange("c o kh kw -> c (o kh kw)").bitcast(F32R))
                    dw0t.append(dw)

                def deconv_chunk(inp_t, inp_hw, w_t, cin, mw, mi, out_t, bias_t):
                    hi, wi = inp_hw
                    ip_w = wi + 2
                    op_w = 2 * wi + 2
                    for py in range(2):
                        for px in range(2):
                            ps = prps.tile([128, 16 * 24], F32, tag="dps")
                            first = True
                            taps = [(u, v) for u in range(py, 5, 2) for v in range(px, 5, 2)]
                            for ti, (u, v) in enumerate(taps):
                                dy = (py + 2 - u) // 2
                                dx = (px + 2 - v) // 2
                                for ci in range(cdiv(cin, 128)):
                                    lhsT = _ap(w_t, ci * 128 * 25, u * 5 + v,
                                               [[2 * 128 * 25, 128], [25, mw]])
                                    rhs = _ap(inp_t, ci * (hi + 2) * ip_w,
                                              (1 + dy) * ip_w + (1 + dx),
                                              [[2 * (hi + 2) * ip_w, 128], [ip_w, hi], [1, wi]])
                                    last = (ti == len(taps) - 1) and (ci == cdiv(cin, 128) - 1)
                                    nc.tensor.matmul(ps[0:mw, 0:hi * wi], lhsT, rhs,
                                                     start=first, stop=last,
                                                     skip_group_check=True)
                                    first = False
                            dst = _ap(out_t, mi * (2 * hi + 2) * op_w,
                                      (py + 1) * op_w + (px + 1),
                                      [[out_t.shape[1] * (2 * hi + 2) * op_w, mw],
                                       [2 * op_w, hi], [2, wi]])
                            nc.scalar.activation(
                                dst, ps[0:mw, 0:hi * wi].rearrange("p (a b) -> p a b", a=hi),
                                mybir.ActivationFunctionType.Lrelu,
                                bias=bias_t[0:mw, mi][:, None], alpha=0.01)

                # deconv0: z[192,8,12] -> m1[192,16,24]
                for mi, (ms, mw) in enumerate(chunks_of(192)):
                    deconv_chunk(zp, (8, 12), dw0t[mi], 192, mw, mi, m1, b_d0)

                # deconv1: m1[192,16,24] -> m2[288,32,48]
                for mi, (ms, mw) in enumerate(chunks_of(288)):
                    dw = pro.tile([128, 2, 128 * 25], F32R, tag="dw", name="dw", bufs=3)
                    nc.gpsimd.memset(dw[64:128, 1, :].bitcast(F32), 0.0)
                    for ci, (cs, cww) in enumerate(chunks_of(192)):
                        nc.sync.dma_start(
                            dw[0:cww, ci, 0:mw * 25],
                            di['hs_dw1'].ap()[cs:cs + cww, ms:ms + mw]
                            .rearrange("c o kh kw -> c (o kh kw)").bitcast(F32R))
                    deconv_chunk(m1, (16, 24), dw, 192, mw, mi, m2, b_d1)

            # ============ PROLOGUE 2: weight transposes ============
            with tc.tile_pool(name="pro", bufs=2) as pro, \
                 tc.tile_pool(name="prps", bufs=2, space="PSUM") as prps:

                tp_count = [0]
                def evict(dst_ap, src_ap):
                    if tp_count[0] % 2 == 0:
                        nc.vector.tensor_copy(dst_ap, src_ap)
                    else:
                        nc.scalar.activation(dst_ap, src_ap,
                                             mybir.ActivationFunctionType.Copy)
                    tp_count[0] += 1

                # DMAs for this phase, priority order: ep_w0, ctx_w, cw2 (2 of 3)
                wnat0 = pro.tile([128, 5, 768], F32, tag="wnat0", bufs=1)
                for mi, (ms, mw) in enumerate(chunks_of(640)):
                    nc.sync.dma_start(wnat0[0:mw, mi, 0:768], di['ep_w0'].ap()[ms:ms + mw])
                cwn = [pro.tile([128, 192 * 25], F32R, tag=f"cwn{mi}", bufs=1, name=f"cwn{mi}")
                       for mi in range(3)]
                for mi in range(3):
                    nc.sync.dma_start(
                        cwn[mi][:],
                        di['ctx_w'].ap()[mi * 128:(mi + 1) * 128]
                        .rearrange("o c kh kw -> o (c kh kw)").bitcast(F32R))
                cw2st = []
                for mi in range(1):
                    cw2s = pro.tile([128, 288 * 9], F32, tag="cw2s", name="cw2s", bufs=1)
                    nc.sync.dma_start(
                        cw2s[:],
                        di['hs_cw2'].ap()[mi * 128:(mi + 1) * 128]
                        .rearrange("o c kh kw -> o (c kh kw)"))
                    cw2st.append(cw2s)

                W0cT32 = pro.tile([128, 3, 640], F32R, tag="w0c32", bufs=1)

                def transpose_batch(srcs, dsts):
                    """srcs: list of (src_ap, pr, fr) transposed into one psum tile;
                    dsts: list of (dst_ap) one per src (evicted in one op if the
                    caller passes a single merged dst as dsts=[(merged, total)])."""
                    pt = prps.tile([128, 4, 128], F32, tag="tp")
                    for k, (sap, pr, fr) in enumerate(srcs):
                        nc.tensor.transpose(pt[0:pr, k, 0:fr], sap, ident[0:fr, 0:fr])
                    return pt

                # ep_w0 -> W0fT / W0cT (+f32r copy of ctx half)
                for ci in range(6):
                    cww = 128
                    for mb in range(2):   # out chunks [0..4) batched 4 + 1
                        mchunks = list(enumerate(chunks_of(640)))[mb * 4:(mb + 1) * 4]
                        if not mchunks:
                            continue
                        pt = prps.tile([128, 4, 128], F32, tag="tp")
                        for k, (mi, (ms, mw)) in enumerate(mchunks):
                            nc.tensor.transpose(pt[0:cww, k, 0:mw],
                                                wnat0[0:mw, mi, ci * 128:ci * 128 + cww],
                                                ident[0:mw, 0:mw])
                        ms0 = mchunks[0][1][0]
                        tw = sum(mw for _, (ms, mw) in mchunks)
                        span = pt[:, 0:len(mchunks), :].rearrange("p a b -> p (a b)")[0:cww, 0:tw]
                        if ci < 3:
                            evict(W0fT[0:cww, ci, ms0:ms0 + tw], span)
                        else:
                            nc.vector.tensor_copy(W0cT[0:cww, ci - 3, ms0:ms0 + tw], span)
                            nc.scalar.activation(W0cT32[0:cww, ci - 3, ms0:ms0 + tw], span,
                                                 mybir.ActivationFunctionType.Copy)

                # ctx_w -> WdT (old taps), batched 3 per evict
                for d in OLD_TAPS:
                    ky, kx = 2 - d[0], 2 - d[1]
                    for ci, (cs, cww) in enumerate(chunks_of(192)):
                        pt = prps.tile([128, 4, 128], F32, tag="tp")
                        for mi in range(3):
                            sap = _ap(cwn[mi], 0, cs * 25 + ky * 5 + kx,
                                      [[192 * 25, 128], [25, cww]]).bitcast(F32)
                            nc.tensor.transpose(pt[0:cww, mi, 0:128], sap, ident[:])
                        span = pt[:, 0:3, :].rearrange("p a b -> p (a b)")[0:cww, 0:384]
                        evict(WdT[d][0:cww, ci, 0:384], span)

                # ep_w1..5 -> WT (wnat rotation overlaps WdT PE work above)
                def load_and_transpose(li):
                    n_out, n_in = DIMS[li + 1], DIMS[li]
                    wnat = pro.tile([128, 4, 640], F32, tag="wnatS", name="wnatS", bufs=1)
                    for mi, (ms, mw) in enumerate(chunks_of(n_out)):
                        nc.sync.dma_start(wnat[0:mw, mi, 0:n_in],
                                          di[f'ep_w{li}'].ap()[ms:ms + mw])
                    for ci, (cs, cww) in enumerate(chunks_of(n_in)):
                        mchunks = list(enumerate(chunks_of(n_out)))
                        for mb in range(cdiv(len(mchunks), 4)):
                            mcb = mchunks[mb * 4:(mb + 1) * 4]
                            pt = prps.tile([128, 4, 128], F32, tag="tp")
                            for k, (mi, (ms, mw)) in enumerate(mcb):
                                nc.tensor.transpose(pt[0:cww, k, 0:mw],
                                                    wnat[0:mw, mi, cs:cs + cww],
                                                    ident[0:mw, 0:mw])
                            ms0 = mcb[0][1][0]
                            tw = sum(mw for _, (ms, mw) in mcb)
                            span = pt[:, 0:len(mcb), :].rearrange("p a b -> p (a b)")[0:cww, 0:tw]
                            evict(WT[li][0:cww, ci, ms0:ms0 + tw], span)
                for li in range(1, 6):
                    load_and_transpose(li)

                # GT[f] = (W0c @ Wd_tap)^T
                for f, d in enumerate(FRESH_TAPS):
                    ky, kx = 2 - d[0], 2 - d[1]
                    for mc, (cs, cww) in enumerate(chunks_of(192)):
                        for nh in range(2):   # 640 -> 2 x 320
                            gp = prps.tile([128, 384], F32, tag="dps")
                            for ki in range(3):
                                lhsT = _ap(cwn[ki], 0, cs * 25 + ky * 5 + kx,
                                           [[192 * 25, 128], [25, cww]])
                                rhs = W0cT32[0:128, ki, nh * 320:(nh + 1) * 320]
                                nc.tensor.matmul(gp[0:cww, 0:320], lhsT, rhs,
                                                 start=(ki == 0), stop=(ki == 2),
                                                 skip_group_check=True)
                            evict(GT[f][0:cww, mc, nh * 320:(nh + 1) * 320],
                                  gp[0:cww, 0:320])

                # conv2 weights -> cw2T [cin, mi, si, k*128+o], batched evicts
                for mi in range(3):
                    if mi >= 1:
                        cw2s = pro.tile([128, 288 * 9], F32, tag="cw2s", name="cw2s", bufs=1)
                        nc.sync.dma_start(
                            cw2s[:],
                            di['hs_cw2'].ap()[mi * 128:(mi + 1) * 128]
                            .rearrange("o c kh kw -> o (c kh kw)"))
                    else:
                        cw2s = cw2st[mi]
                    nc.vector.memset(cw2T[32:64, mi, 2, :], 0.0)
                    nc.gpsimd.memset(cw2T[64:128, mi, 2, :], 0.0)
                    for si, (ss, sw) in enumerate(chunks_of(288)):
                        for kb in range(3):   # k batched 4+4+1
                            ks = list(range(9))[kb * 4:(kb + 1) * 4]
                            if not ks:
                                continue
                            pt = prps.tile([128, 4, 128], F32, tag="tp")
                            for kk, k in enumerate(ks):
                                sap = _ap(cw2s, 0, ss * 9 + k, [[288 * 9, 128], [9, sw]])
                                nc.tensor.transpose(pt[0:sw, kk, 0:128], sap, ident[:])
                            span = pt[:, 0:len(ks), :].rearrange("p a b -> p (a b)")[0:sw, 0:len(ks) * 128]
                            evict(_ap(cw2T, (mi * 3 + si) * 9 * 128, ks[0] * 128,
                                      [[3 * 3 * 9 * 128, sw], [1, len(ks) * 128]]),
                                  span)

            # conv2 rows 0..1 upfront; rest streamed into the wavefront steps
            for mi in range(3):
                emit_conv2_unit(nc, pps, cw2T, m2, fm1, b_c2, mi, 0, 1, 0, 27)

            # ================= SEQUENTIAL PHASE =================
            # conv2 streaming: 2-row units (mi, rb), rows [2rb, 2rb+2)
            pf_units = [(mi, rb) for rb in range(1, 16) for mi in range(3)]
            pf_state = {"u": 0, "k": 0, "ps": None}

            def pfill(nmm):
                while nmm > 0 and pf_state["u"] < len(pf_units):
                    mi, rb = pf_units[pf_state["u"]]
                    take = min(nmm, 27 - pf_state["k"])
                    ps = emit_conv2_unit(nc, pps, cw2T, m2, fm1, b_c2, mi,
                                         2 * rb, 2 * rb + 2,
                                         pf_state["k"], pf_state["k"] + take,
                                         ps=pf_state["ps"])
                    pf_state["ps"] = ps
                    pf_state["k"] += take
                    nmm -= take
                    if pf_state["k"] == 27:
                        pf_state["u"] += 1
                        pf_state["k"] = 0
                        pf_state["ps"] = None

            def ydiag_ap(img, i0, j0, kw, c, B):
                """[kw, B] wavefront-diagonal AP into padded img tile chunk c."""
                off = (i0 + 2) * WP + (j0 + 2)
                return _ap(img, c * HP * WP, off, [[2 * HP * WP, kw], [DIAG, B]])

            def emit_seed2(pt, slot, brow_ap, mw, B):
                nc.tensor.matmul(pt[0:mw, slot, 0:B], brow_ap, ones[0:1, 0:B],
                                 start=True, stop=False, skip_group_check=True)

            def emit_old_ctx(t1):
                """ctx_b seed for step t1's ctx -> zc[t1%2][5:8]; return tap MM list."""
                s1 = t1 % 2
                i_lo, B, j_lo = step_geom(t1)
                ms_list = chunks_of(384)
                for m, (ms, mw) in enumerate(ms_list):
                    emit_seed2(zc[s1], 5 + m, ctxb[0:1, ms:ms + mw], mw, B)
                mms = []
                for ti, (dy, dx) in enumerate(OLD_TAPS):
                    for c, (cs, kw) in enumerate(chunks_of(192)):
                        for m, (ms, mw) in enumerate(ms_list):
                            mms.append((ti, dy, dx, c, cs, kw, m, ms, mw))
                return i_lo, B, j_lo, mms

            # prologue part of step 0's z0/ctx accumulation
            def emit_z0_pre(t1):
                """seeds + f-part + (later) ctx-part for z0 of step t1 -> zc[t1%2][0:5]"""
                s1 = t1 % 2
                i_lo, B, j_lo = step_geom(t1)
                for m, (ms, mw) in enumerate(chunks_of(640)):
                    emit_seed2(zc[s1], m, brow[0][0:1, ms:ms + mw], mw, B)
                for k in range(3):
                    for m, (ms, mw) in enumerate(chunks_of(640)):
                        rhs = _ap(fm1, k * NPIX, i_lo * W + j_lo,
                                  [[3 * NPIX, 128], [W - 3, B]])
                        nc.tensor.matmul(zc[s1][0:mw, m, 0:B],
                                         W0fT[0:128, k, ms:ms + mw], rhs,
                                         start=False, stop=False, skip_group_check=True)

            def emit_z0_ctx(t1):
                s1 = t1 % 2
                i_lo, B, j_lo = step_geom(t1)
                for k in range(3):
                    for m, (ms, mw) in enumerate(chunks_of(640)):
                        nc.tensor.matmul(zc[s1][0:mw, m, 0:B],
                                         W0cT[0:128, k, ms:ms + mw],
                                         Xc[0:128, k, 0:B],
                                         start=False, stop=False, skip_group_check=True)

            def emit_old_mms(t1, geom, mms):
                i_lo, B, j_lo = geom
                s1 = t1 % 2
                for (ti, dy, dx, c, cs, kw, m, ms, mw) in mms:
                    rhs = ydiag_ap(Yimg, i_lo - dy, j_lo - dx, kw, c, B)
                    last = (ti == len(OLD_TAPS) - 1) and (c == 1)
                    nc.tensor.matmul(zc[s1][0:mw, 5 + m, 0:B],
                                     WdT[OLD_TAPS[ti]][0:kw, c, ms:ms + mw], rhs,
                                     start=False, stop=last, skip_group_check=True)

            # --- step 0 pre-work (its sources are all zero borders) ---
            g0 = emit_old_ctx(0)
            emit_old_mms(0, (g0[0], g0[1], g0[2]), g0[3])
            i_lo0, B0, j_lo0 = step_geom(0)
            nc.vector.tensor_copy(Xc[:, 0:3, 0:B0], zc[0][:, 5:8, 0:B0])
            emit_z0_pre(0)
            emit_z0_ctx(0)

            KCHW = {li: chunks_of(DIMS[li]) for li in range(1, 6)}
            MCHW = {li: chunks_of(DIMS[li + 1]) for li in range(0, 6)}

            for t in range(nsteps):
                s = t % 2
                s1 = (t + 1) % 2
                i_lo, B, j_lo = step_geom(t)
                have_next = t + 1 < nsteps
                if have_next:
                    i_lo1, B1, j_lo1 = step_geom(t + 1)

                # ---- fresh taps -> z0 (critical) ----
                for m, (ms, mw) in enumerate(MCHW[0]):
                    for f in range(2):
                        dy, dx = FRESH_TAPS[f]
                        for c, (cs, kw) in enumerate(chunks_of(192)):
                            rhs = ydiag_ap(Yimg, i_lo - dy, j_lo - dx, kw, c, B)
                            nc.tensor.matmul(zc[s][0:mw, m, 0:B],
                                             GT[f][0:kw, c, ms:ms + mw], rhs,
                                             start=False,
                                             stop=(f == 1 and c == 1),
                                             skip_group_check=True)

                # ---- lrelu z0 -> x1 (critical ACT) ----
                nc.scalar.activation(xs[1][:, 0:5, 0:B], zc[s][:, 0:5, 0:B],
                                     mybir.ActivationFunctionType.Lrelu, alpha=0.01)

                # old-ctx for t+1 (fillers)
                old_mms = []
                if have_next:
                    g = emit_old_ctx(t + 1)
                    old_mms = g[3]
                    geom1 = (g[0], g[1], g[2])

                # ---- MLP layers 1..5 ----
                for li in range(1, 6):
                    # fillers before the critical MMs of this layer
                    if li == 2 and have_next:
                        emit_old_mms(t + 1, geom1, old_mms[:33])
                    elif li == 3 and have_next:
                        emit_old_mms(t + 1, geom1, old_mms[33:])
                    elif li == 4 and have_next:
                        nc.vector.tensor_copy(Xc[:, 0:3, 0:B1], zc[s1][:, 5:8, 0:B1])
                        emit_z0_pre(t + 1)
                        pfill(10)
                    elif li == 5 and have_next:
                        pfill(14)

                    # seed + main MMs -> zs
                    for m, (ms, mw) in enumerate(MCHW[li]):
                        emit_seed2(zs, ZOFF[li] + m, brow[li][0:1, ms:ms + mw], mw, B)
                    kch = KCHW[li]
                    for m, (ms, mw) in enumerate(MCHW[li]):
                        for k, (ks, kw) in enumerate(kch):
                            nc.tensor.matmul(zs[0:mw, ZOFF[li] + m, 0:B],
                                             WT[li][0:kw, k, ms:ms + mw],
                                             xs[li][0:kw, k, 0:B],
                                             start=False, stop=(k == len(kch) - 1),
                                             skip_group_check=True)
                    if li < 5:
                        nch = len(MCHW[li])
                        nc.scalar.activation(xs[li + 1][:, 0:nch, 0:B],
                                             zs[:, ZOFF[li]:ZOFF[li] + nch, 0:B],
                                             mybir.ActivationFunctionType.Lrelu,
                                             alpha=0.01)

                # ---- Y = z5 + w_hat (critical DVE) ----
                off = (i_lo + 2) * WP + (j_lo + 2)
                ydst = _ap(Yimg, 0, off, [[2 * HP * WP, 128], [HP * WP, 2], [DIAG, B]])
                ywim = _ap(wimg, 0, off, [[2 * HP * WP, 128], [HP * WP, 2], [DIAG, B]])
                nc.vector.tensor_tensor(ydst, zs[:, 12:14, 0:B], ywim,
                                        mybir.AluOpType.add)
                # late filler: ctx->z0 for t+1 (needs Xc evict from ~2 hops ago)
                if have_next:
                    emit_z0_ctx(t + 1)

            # ================= EPILOGUE =================
            with tc.tile_pool(name="epi", bufs=1) as epi:
                Yimg32 = epi.tile([128, 2, NPIX], F32)
                src = _ap(Yimg, 0, 2 * WP + 2,
                          [[2 * HP * WP, 128], [HP * WP, 2], [WP, H], [1, W]])
                dst = _ap(Yimg32, 0, 0,
                          [[2 * NPIX, 128], [NPIX, 2], [W, H], [1, W]])
                nc.vector.tensor_copy(dst, src)
                ov = out.ap()[0]
                for ci, (cs, cw) in enumerate(chunks_of(192)):
                    nc.sync.dma_start(
                        ov[cs:cs + cw],
                        Yimg32[0:cw, ci, :].rearrange("p (h w) -> p h w", h=H))

    nc.compile()
    return nc


def emit_conv2_unit(nc, pps, cw2T, m2, fm1, b_c2, mi, r0, r1, k0, k1, ps=None):
    """Emit conv2 MMs [k0, k1) for out-chunk mi over rows [r0, r1);
    27 MMs per unit. MM index kk = k * 3 + si."""
    F32 = mybir.dt.float32
    nr = r1 - r0
    if ps is None:
        ps = pps.tile([128, 384], F32, tag="cps", name="cps")
    for kk in range(k0, k1):
        k, si = kk // 3, kk % 3
        ky, kx = k // 3, k % 3
        lhsT = _ap(cw2T, (mi * 3 + si) * 9 * 128, k * 128,
                   [[3 * 3 * 9 * 128, 128], [1, 128]])
        rhs = _ap(m2, si * 34 * 50, (ky + r0) * 50 + kx,
                  [[3 * 34 * 50, 128], [50, nr], [1, 48]])
        nc.tensor.matmul(ps[:, 0:nr * 48], lhsT, rhs,
                         start=(kk == 0), stop=(kk == 26), skip_group_check=True)
    if k1 == 27:
        nc.scalar.activation(fm1[:, mi, r0 * 48:r1 * 48], ps[:, 0:nr * 48],
                             mybir.ActivationFunctionType.Identity,
                             bias=b_c2[:, mi][:, None], alpha=0.0)
    return ps


_NC_CACHE = {}


def kernel(**inputs):
    from concourse.bass_utils import run_bass_kernel_spmd
    key = "full"
    if key not in _NC_CACHE:
        _NC_CACHE[key] = build()
    nc = _NC_CACHE[key]
    in_map = {k: np.ascontiguousarray(np.asarray(v, dtype=np.float32))
              for k, v in inputs.items()}
    res = run_bass_kernel_spmd(nc, [in_map] * 8, core_ids=list(range(8)))
    return res.results[0]['out']


if __name__ == "__main__":
    t = build(nsteps=int(sys.argv[1]) if len(sys.argv) > 1 else NSTEPS)
    print("build ok")
    from concourse.timeline_sim import TimelineSim
    est = TimelineSim(t).simulate()
    print(f"HW exec time: {est:.0f} ns")


# revision 23
# speedup vs baseline: 4.6772x; 1.8466x over previous
"""Trainium2 Bass kernel for nn_BEE_Bin2Symbol (hyper-decoder + masked-conv
autoregressive MLP decoder).

Architecture (v2, latency-oriented):
- Sequential phase runs all GEMMs in [C_out-on-partitions, B-pixels-free]
  orientation (weights stationary as lhsT, activations moving): matmul cost
  scales with B<=16, transposes disappear, and each layer's nonlinearity is a
  single DVE scalar_tensor_tensor op  max(0.01*z, z)  reading PSUM directly.
- 140 slope-3 anti-diagonal wavefronts. Per step the critical chain is
  fresh-tap MMs -> lrelu(z0) -> W1 MMs -> lrelu(z1) -> ... -> z5 -> Y-add.
  Everything else (old-tap ctx GEMMs, f-part, bias seeds, phase-P conv
  streaming) is issued into the PE gaps between the chain's engine hops.
- Fresh taps (age-1) use precomposed G = W0c @ Wd_tap so they feed z0
  directly; old taps accumulate a ctx PSUM a step ahead, evicted by the
  scalar engine (ACT) off the critical path.
- All sequential-phase weights/activations in bf16 (PSUM accumulate f32).
- Hyper-decoder (2 stride-2 deconvs + 3x3 conv) in f32r as phase-decomposed
  GEMMs; conv2's last 3 row-blocks stream into the first ~15 wavefront steps.
"""
import sys, os
sys.path.insert(0, "/opt/trn_rl_repo")

import numpy as np

import concourse.bass as bass
import concourse.bacc as bacc
import concourse.mybir as mybir
import concourse.tile as tile
from concourse.masks import make_identity

F32 = mybir.dt.float32
F32R = mybir.dt.float32r
BF16 = mybir.dt.bfloat16

H, W = 32, 48
HP, WP = H + 4, W + 4            # padded image 36 x 52
NPIX = H * W
NSTEPS = 3 * (H - 1) + (W - 1) + 1   # 140
DIAG = WP - 3                    # 49: wavefront-diagonal stride in padded img

# taps (dy, dx): source pixel = (i-dy, j-dx); ctx_w index (ky,kx) = (2-dy, 2-dx)
TAPS = [(2, 2), (2, 1), (2, 0), (2, -1), (2, -2),
        (1, 2), (1, 1), (1, 0), (1, -1), (1, -2),
        (0, 1), (0, 2)]
FRESH_TAPS = [(1, -2), (0, 1)]                # age-1 taps (need step t-1)
OLD_TAPS = [d for d in TAPS if d not in FRESH_TAPS]

DIMS = [768, 640, 512, 384, 320, 256, 192]   # MLP dims; layer l: DIMS[l]->DIMS[l+1]
# z1..z5 chunk offsets inside the shared zs psum tile [128, 14, 16]
ZOFF = {1: 0, 2: 4, 3: 7, 4: 10, 5: 12}


def cdiv(a, b):
    return (a + b - 1) // b


def chunks_of(n, c=128):
    return [(s, min(c, n - s)) for s in range(0, n, c)]


def _ap(tile_ap, slot_off, elem_off, plist):
    """Build a custom AP into a [128, S, F]-shaped sbuf/psum tile."""
    base = tile_ap[:]
    return bass.AP(base.tensor, base.offset + slot_off + elem_off, plist)


def step_geom(t):
    i_lo = max(0, cdiv(t - (W - 1), 3))
    i_hi = min(H - 1, t // 3)
    return i_lo, i_hi - i_lo + 1, t - 3 * i_lo


def build(nsteps=NSTEPS):
    nc = bacc.Bacc()

    # ---------------- DRAM I/O ----------------
    di = {}
    di['z_hat'] = nc.dram_tensor('z_hat', [1, 192, 8, 12], F32, kind="ExternalInput")
    di['w_hat'] = nc.dram_tensor('w_hat', [1, 192, 32, 48], F32, kind="ExternalInput")
    di['hs_dw0'] = nc.dram_tensor('hs_dw0', [192, 192, 5, 5], F32, kind="ExternalInput")
    di['hs_db0'] = nc.dram_tensor('hs_db0', [192], F32, kind="ExternalInput")
    di['hs_dw1'] = nc.dram_tensor('hs_dw1', [192, 288, 5, 5], F32, kind="ExternalInput")
    di['hs_db1'] = nc.dram_tensor('hs_db1', [288], F32, kind="ExternalInput")
    di['hs_cw2'] = nc.dram_tensor('hs_cw2', [384, 288, 3, 3], F32, kind="ExternalInput")
    di['hs_cb2'] = nc.dram_tensor('hs_cb2', [384], F32, kind="ExternalInput")
    di['ctx_w'] = nc.dram_tensor('ctx_w', [384, 192, 5, 5], F32, kind="ExternalInput")
    di['ctx_b'] = nc.dram_tensor('ctx_b', [384], F32, kind="ExternalInput")
    for li in range(6):
        di[f'ep_w{li}'] = nc.dram_tensor(f'ep_w{li}', [DIMS[li + 1], DIMS[li]], F32,
                                         kind="ExternalInput")
        di[f'ep_b{li}'] = nc.dram_tensor(f'ep_b{li}', [DIMS[li + 1]], F32,
                                         kind="ExternalInput")
    out = nc.dram_tensor('out', [1, 192, 32, 48], F32, kind="ExternalOutput")

    with tile.TileContext(nc) as tc:
        with tc.tile_pool(name="pp", bufs=1) as pp, \
             tc.tile_pool(name="pps", bufs=1, space="PSUM") as pps:

            ident = pp.tile([128, 128], F32)
            make_identity(nc, ident[:])

            # ---------- persistent state ----------
            Yimg = pp.tile([128, 2, HP * WP], BF16)   # decoded image (padded)
            nc.vector.memset(Yimg[:], 0.0)
            wimg = pp.tile([128, 2, HP * WP], F32)    # w_hat residual (padded)
            nc.gpsimd.memset(wimg[:], 0.0)
            fm1 = pp.tile([128, 3, NPIX], BF16)       # conv2 output [384, 1536]
            m2 = pp.tile([128, 3, 34 * 50], BF16)     # deconv1 out (padded 34x50)
            nc.gpsimd.memset(m2[:], 0.0)
            cw2T = pp.tile([128, 3, 3, 9 * 128], BF16)  # [cin, mi, si, k*128+o]

            # transposed weights (bf16)
            W0fT = pp.tile([128, 3, 640], BF16)
            W0cT = pp.tile([128, 3, 640], BF16)
            WT = {}
            for li in range(1, 6):
                WT[li] = pp.tile([128, cdiv(DIMS[li], 128), DIMS[li + 1]], BF16,
                                 tag=f"W{li}T", name=f"W{li}T")
            GT = [pp.tile([128, 2, 640], BF16, tag=f"GT{k}", name=f"GT{k}") for k in range(2)]
            WdT = {}
            for d in OLD_TAPS:
                WdT[d] = pp.tile([128, 2, 384], BF16, tag=f"Wd{d[0]}_{d[1]}", name=f"Wd{d[0]}_{d[1]}")

            # bias rows (lhsT for K=1 seed matmuls)
            brow = [pp.tile([1, DIMS[li + 1]], BF16, tag=f"b{li}", name=f"b{li}") for li in range(6)]
            ctxb = pp.tile([1, 384], BF16)
            ones = pp.tile([1, 16], BF16)
            nc.vector.memset(ones[:], 1.0)

            # sequential-phase activations (single tiles; WAR handled by sems)
            xs = {li: pp.tile([128, cdiv(DIMS[li], 128), 16], BF16, tag=f"x{li}", name=f"x{li}")
                  for li in range(1, 6)}
            Xc = pp.tile([128, 3, 16], BF16)          # evicted old-ctx

            # persistent psum: z0+ctx ring (2 banks), mlp zs (1), conv stream (1)
            zc = [pps.tile([128, 8, 16], F32, tag=f"zc{s}", name=f"zc{s}") for s in range(2)]
            zs = pps.tile([128, 14, 16], F32, tag="zs")
            nc.vector.memset(zs[:], 0.0)
            nc.vector.memset(zc[0][:], 0.0)
            nc.vector.memset(zc[1][:], 0.0)

            # ============ PROLOGUE 1: hyper-decoder (DMA priority) ============
            with tc.tile_pool(name="proB", bufs=2) as pro, \
                 tc.tile_pool(name="prpsB", bufs=2, space="PSUM") as prps:

                # small DMAs first
                whv = di['w_hat'].ap()[0]
                for ci, (s, cw) in enumerate(chunks_of(192)):
                    dst = _ap(wimg, ci * HP * WP, 2 * WP + 2,
                              [[2 * HP * WP, cw], [WP, H], [1, W]])
                    nc.sync.dma_start(dst, whv[s:s + cw])

                def load_bias_col(name, n):
                    nch = cdiv(n, 128)
                    t = pp.tile([128, nch], F32, tag=f"b_{name}", name=f"b_{name}")
                    nc.vector.memset(t[:], 0.0)
                    for ci, (s, w_) in enumerate(chunks_of(n)):
                        nc.sync.dma_start(t[0:w_, ci:ci + 1], di[name][s:s + w_][:, None])
                    return t
                b_d0 = load_bias_col('hs_db0', 192)
                b_d1 = load_bias_col('hs_db1', 288)
                b_c2 = load_bias_col('hs_cb2', 384)

                def load_brow(dst, dram, n):
                    st = pro.tile([1, 640], F32, tag="brs", name="brs", bufs=3)
                    nc.sync.dma_start(st[0:1, 0:n], dram.ap()[None, :])
                    nc.vector.tensor_copy(dst[0:1, 0:n], st[0:1, 0:n])
                for li in range(6):
                    load_brow(brow[li], di[f'ep_b{li}'], DIMS[li + 1])
                load_brow(ctxb, di['ctx_b'], 384)

                # deconv input + weights (DMAs lead the queue); deconv0 in bf16
                zps = pro.tile([128, 2, 10 * 14], F32, tag="zps", bufs=1)
                zv = di['z_hat'].ap()[0]
                for ci, (s, cw) in enumerate(chunks_of(192)):
                    dst = _ap(zps, ci * 140, 14 + 1, [[2 * 140, cw], [14, 8], [1, 12]])
                    nc.sync.dma_start(dst, zv[s:s + cw])
                zp = pro.tile([128, 2, 10 * 14], BF16, tag="zp", bufs=1)
                nc.vector.memset(zp[:], 0.0)
                for ci, (cs, cww) in enumerate(chunks_of(192)):
                    nc.vector.tensor_copy(zp[0:cww, ci, :], zps[0:cww, ci, :])

                m1 = pro.tile([128, 2, 18 * 26], F32R, tag="m1", bufs=1)
                nc.vector.memset(m1[:].bitcast(F32), 0.0)

                cvt_eng = [nc.scalar, nc.gpsimd, nc.vector]
                dw0t = []
                for mi, (ms, mw) in enumerate(chunks_of(192)):
                    dws = pro.tile([128, 2, 128 * 25], F32, tag="dw", name="dw", bufs=2)
                    for ci, (cs, cww) in enumerate(chunks_of(192)):
                        nc.sync.dma_start(
                            dws[0:cww, ci, 0:mw * 25],
                            di['hs_dw0'].ap()[cs:cs + cww, ms:ms + mw]
                            .rearrange("c o kh kw -> c (o kh kw)"))
                    dwb = pro.tile([128, 2, 128 * 25], BF16, tag="dwb", name="dwb", bufs=2)
                    for ci, (cs, cww) in enumerate(chunks_of(192)):
                        if mi == 0:
                            nc.scalar.activation(dwb[0:cww, ci, 0:mw * 25],
                                                 dws[0:cww, ci, 0:mw * 25],
                                                 mybir.ActivationFunctionType.Copy)
                        elif mi == 1:
                            nc.gpsimd.tensor_copy(dwb[0:cww, ci, 0:mw * 25],
                                                  dws[0:cww, ci, 0:mw * 25])
                        else:
                            nc.vector.tensor_copy(dwb[0:cww, ci, 0:mw * 25],
                                                  dws[0:cww, ci, 0:mw * 25])
                    dw0t.append(dwb)

                def deconv_chunk(inp_t, inp_hw, w_t, cin, mw, mi, out_t, bias_t):
                    hi, wi = inp_hw
                    ip_w = wi + 2
                    op_w = 2 * wi + 2
                    for py in range(2):
                        for px in range(2):
                            ps = prps.tile([128, 16 * 24], F32, tag="dps")
                            first = True
                            taps = [(u, v) for u in range(py, 5, 2) for v in range(px, 5, 2)]
                            for ti, (u, v) in enumerate(taps):
                                dy = (py + 2 - u) // 2
                                dx = (px + 2 - v) // 2
                                for ci, (cs, cww) in enumerate(chunks_of(cin)):
                                    lhsT = _ap(w_t, ci * 128 * 25, u * 5 + v,
                                               [[2 * 128 * 25, cww], [25, mw]])
                                    rhs = _ap(inp_t, ci * (hi + 2) * ip_w,
                                              (1 + dy) * ip_w + (1 + dx),
                                              [[2 * (hi + 2) * ip_w, cww], [ip_w, hi], [1, wi]])
                                    last = (ti == len(taps) - 1) and (ci == len(chunks_of(cin)) - 1)
                                    nc.tensor.matmul(ps[0:mw, 0:hi * wi], lhsT, rhs,
                                                     start=first, stop=last,
                                                     skip_group_check=True)
                                    first = False
                            dst = _ap(out_t, mi * (2 * hi + 2) * op_w,
                                      (py + 1) * op_w + (px + 1),
                                      [[out_t.shape[1] * (2 * hi + 2) * op_w, mw],
                                       [2 * op_w, hi], [2, wi]])
                            nc.scalar.activation(
                                dst, ps[0:mw, 0:hi * wi].rearrange("p (a b) -> p a b", a=hi),
                                mybir.ActivationFunctionType.Lrelu,
                                bias=bias_t[0:mw, mi][:, None], alpha=0.01)

                # deconv0: z[192,8,12] -> m1[192,16,24]
                for mi, (ms, mw) in enumerate(chunks_of(192)):
                    deconv_chunk(zp, (8, 12), dw0t[mi], 192, mw, mi, m1, b_d0)

                # deconv1: m1[192,16,24] -> m2[288,32,48]
                for mi, (ms, mw) in enumerate(chunks_of(288)):
                    dw = pro.tile([128, 2, 128 * 25], F32R, tag="dw", name="dw1", bufs=2)
                    for ci, (cs, cww) in enumerate(chunks_of(192)):
                        nc.scalar.dma_start(
                            dw[0:cww, ci, 0:mw * 25],
                            di['hs_dw1'].ap()[cs:cs + cww, ms:ms + mw]
                            .rearrange("c o kh kw -> c (o kh kw)").bitcast(F32R))
                    deconv_chunk(m1, (16, 24), dw, 192, mw, mi, m2, b_d1)

            # ============ PROLOGUE 2: weight transposes ============
            with tc.tile_pool(name="pro", bufs=2) as pro, \
                 tc.tile_pool(name="prps", bufs=2, space="PSUM") as prps:

                tp_count = [0]
                def evict(dst_ap, src_ap):
                    if tp_count[0] % 2 == 0:
                        nc.vector.tensor_copy(dst_ap, src_ap)
                    else:
                        nc.scalar.activation(dst_ap, src_ap,
                                             mybir.ActivationFunctionType.Copy)
                    tp_count[0] += 1

                # DMAs for this phase, priority order: ep_w0, ctx_w, cw2 (2 of 3)
                wnat0 = pro.tile([128, 5, 768], F32, tag="wnat0", bufs=1)
                for mi, (ms, mw) in enumerate(chunks_of(640)):
                    nc.sync.dma_start(wnat0[0:mw, mi, 0:768], di['ep_w0'].ap()[ms:ms + mw])
                cwn = [pro.tile([128, 192 * 25], F32R, tag=f"cwn{mi}", bufs=1, name=f"cwn{mi}")
                       for mi in range(3)]
                for mi in range(3):
                    nc.sync.dma_start(
                        cwn[mi][:],
                        di['ctx_w'].ap()[mi * 128:(mi + 1) * 128]
                        .rearrange("o c kh kw -> o (c kh kw)").bitcast(F32R))
                cw2st = []
                for mi in range(1):
                    cw2s = pro.tile([128, 288 * 9], F32, tag="cw2s", name="cw2s", bufs=1)
                    nc.sync.dma_start(
                        cw2s[:],
                        di['hs_cw2'].ap()[mi * 128:(mi + 1) * 128]
                        .rearrange("o c kh kw -> o (c kh kw)"))
                    cw2st.append(cw2s)

                W0cT32 = pro.tile([128, 3, 640], F32R, tag="w0c32", bufs=1)

                def transpose_batch(srcs, dsts):
                    """srcs: list of (src_ap, pr, fr) transposed into one psum tile;
                    dsts: list of (dst_ap) one per src (evicted in one op if the
                    caller passes a single merged dst as dsts=[(merged, total)])."""
                    pt = prps.tile([128, 4, 128], F32, tag="tp")
                    for k, (sap, pr, fr) in enumerate(srcs):
                        nc.tensor.transpose(pt[0:pr, k, 0:fr], sap, ident[0:fr, 0:fr])
                    return pt

                # ep_w0 -> W0fT / W0cT (+f32r copy of ctx half)
                for ci in range(6):
                    cww = 128
                    for mb in range(2):   # out chunks [0..4) batched 4 + 1
                        mchunks = list(enumerate(chunks_of(640)))[mb * 4:(mb + 1) * 4]
                        if not mchunks:
                            continue
                        pt = prps.tile([128, 4, 128], F32, tag="tp")
                        for k, (mi, (ms, mw)) in enumerate(mchunks):
                            nc.tensor.transpose(pt[0:cww, k, 0:mw],
                                                wnat0[0:mw, mi, ci * 128:ci * 128 + cww],
                                                ident[0:mw, 0:mw])
                        ms0 = mchunks[0][1][0]
                        tw = sum(mw for _, (ms, mw) in mchunks)
                        span = pt[:, 0:len(mchunks), :].rearrange("p a b -> p (a b)")[0:cww, 0:tw]
                        if ci < 3:
                            evict(W0fT[0:cww, ci, ms0:ms0 + tw], span)
                        else:
                            nc.vector.tensor_copy(W0cT[0:cww, ci - 3, ms0:ms0 + tw], span)
                            nc.scalar.activation(W0cT32[0:cww, ci - 3, ms0:ms0 + tw], span,
                                                 mybir.ActivationFunctionType.Copy)

                # ctx_w -> WdT (old taps), batched 3 per evict
                for d in OLD_TAPS:
                    ky, kx = 2 - d[0], 2 - d[1]
                    for ci, (cs, cww) in enumerate(chunks_of(192)):
                        pt = prps.tile([128, 4, 128], F32, tag="tp")
                        for mi in range(3):
                            sap = _ap(cwn[mi], 0, cs * 25 + ky * 5 + kx,
                                      [[192 * 25, 128], [25, cww]]).bitcast(F32)
                            nc.tensor.transpose(pt[0:cww, mi, 0:128], sap, ident[:])
                        span = pt[:, 0:3, :].rearrange("p a b -> p (a b)")[0:cww, 0:384]
                        evict(WdT[d][0:cww, ci, 0:384], span)

                # ep_w1..5 -> WT (wnat rotation overlaps WdT PE work above)
                def load_and_transpose(li):
                    n_out, n_in = DIMS[li + 1], DIMS[li]
                    wnat = pro.tile([128, 4, 640], F32, tag="wnatS", name="wnatS", bufs=1)
                    for mi, (ms, mw) in enumerate(chunks_of(n_out)):
                        nc.gpsimd.dma_start(wnat[0:mw, mi, 0:n_in],
                                            di[f'ep_w{li}'].ap()[ms:ms + mw])
                    for ci, (cs, cww) in enumerate(chunks_of(n_in)):
                        mchunks = list(enumerate(chunks_of(n_out)))
                        for mb in range(cdiv(len(mchunks), 4)):
                            mcb = mchunks[mb * 4:(mb + 1) * 4]
                            pt = prps.tile([128, 4, 128], F32, tag="tp")
                            for k, (mi, (ms, mw)) in enumerate(mcb):
                                nc.tensor.transpose(pt[0:cww, k, 0:mw],
                                                    wnat[0:mw, mi, cs:cs + cww],
                                                    ident[0:mw, 0:mw])
                            ms0 = mcb[0][1][0]
                            tw = sum(mw for _, (ms, mw) in mcb)
                            span = pt[:, 0:len(mcb), :].rearrange("p a b -> p (a b)")[0:cww, 0:tw]
                            evict(WT[li][0:cww, ci, ms0:ms0 + tw], span)
                for li in range(1, 6):
                    load_and_transpose(li)

                # GT[f] = (W0c @ Wd_tap)^T
                for f, d in enumerate(FRESH_TAPS):
                    ky, kx = 2 - d[0], 2 - d[1]
                    for mc, (cs, cww) in enumerate(chunks_of(192)):
                        for nh in range(2):   # 640 -> 2 x 320
                            gp = prps.tile([128, 384], F32, tag="dps")
                            for ki in range(3):
                                lhsT = _ap(cwn[ki], 0, cs * 25 + ky * 5 + kx,
                                           [[192 * 25, 128], [25, cww]])
                                rhs = W0cT32[0:128, ki, nh * 320:(nh + 1) * 320]
                                nc.tensor.matmul(gp[0:cww, 0:320], lhsT, rhs,
                                                 start=(ki == 0), stop=(ki == 2),
                                                 skip_group_check=True)
                            evict(GT[f][0:cww, mc, nh * 320:(nh + 1) * 320],
                                  gp[0:cww, 0:320])

                # conv2 weights -> cw2T [cin, mi, si, k*128+o], batched evicts
                for mi in range(3):
                    if mi >= 1:
                        cw2s = pro.tile([128, 288 * 9], F32, tag="cw2s", name="cw2s", bufs=1)
                        nc.gpsimd.dma_start(
                            cw2s[:],
                            di['hs_cw2'].ap()[mi * 128:(mi + 1) * 128]
                            .rearrange("o c kh kw -> o (c kh kw)"))
                    else:
                        cw2s = cw2st[mi]
                    nc.vector.memset(cw2T[32:64, mi, 2, :], 0.0)
                    nc.gpsimd.memset(cw2T[64:128, mi, 2, :], 0.0)
                    for si, (ss, sw) in enumerate(chunks_of(288)):
                        for kb in range(3):   # k batched 4+4+1
                            ks = list(range(9))[kb * 4:(kb + 1) * 4]
                            if not ks:
                                continue
                            pt = prps.tile([128, 4, 128], F32, tag="tp")
                            for kk, k in enumerate(ks):
                                sap = _ap(cw2s, 0, ss * 9 + k, [[288 * 9, 128], [9, sw]])
                                nc.tensor.transpose(pt[0:sw, kk, 0:128], sap, ident[:])
                            span = pt[:, 0:len(ks), :].rearrange("p a b -> p (a b)")[0:sw, 0:len(ks) * 128]
                            evict(_ap(cw2T, (mi * 3 + si) * 9 * 128, ks[0] * 128,
                                      [[3 * 3 * 9 * 128, sw], [1, len(ks) * 128]]),
                                  span)

            # conv2 rows 0..1 upfront; rest streamed into the wavefront steps
            for mi in range(3):
                emit_conv2_unit(nc, pps, cw2T, m2, fm1, b_c2, mi, 0, 1, 0, 27)

            # ================= SEQUENTIAL PHASE =================
            # conv2 streaming: 2-row units (mi, rb), rows [2rb, 2rb+2)
            pf_units = [(mi, rb) for rb in range(1, 16) for mi in range(3)]
            pf_state = {"u": 0, "k": 0, "ps": None}

            def pfill(nmm):
                while nmm > 0 and pf_state["u"] < len(pf_units):
                    mi, rb = pf_units[pf_state["u"]]
                    take = min(nmm, 27 - pf_state["k"])
                    ps = emit_conv2_unit(nc, pps, cw2T, m2, fm1, b_c2, mi,
                                         2 * rb, 2 * rb + 2,
                                         pf_state["k"], pf_state["k"] + take,
                                         ps=pf_state["ps"])
                    pf_state["ps"] = ps
                    pf_state["k"] += take
                    nmm -= take
                    if pf_state["k"] == 27:
                        pf_state["u"] += 1
                        pf_state["k"] = 0
                        pf_state["ps"] = None

            def ydiag_ap(img, i0, j0, kw, c, B):
                """[kw, B] wavefront-diagonal AP into padded img tile chunk c."""
                off = (i0 + 2) * WP + (j0 + 2)
                return _ap(img, c * HP * WP, off, [[2 * HP * WP, kw], [DIAG, B]])

            def emit_seed2(pt, slot, brow_ap, mw, B):
                nc.tensor.matmul(pt[0:mw, slot, 0:B], brow_ap, ones[0:1, 0:B],
                                 start=True, stop=False, skip_group_check=True)

            def emit_old_ctx(t1):
                """ctx_b seed for step t1's ctx -> zc[t1%2][5:8]; return tap MM list."""
                s1 = t1 % 2
                i_lo, B, j_lo = step_geom(t1)
                ms_list = chunks_of(384)
                for m, (ms, mw) in enumerate(ms_list):
                    emit_seed2(zc[s1], 5 + m, ctxb[0:1, ms:ms + mw], mw, B)
                mms = []
                for ti, (dy, dx) in enumerate(OLD_TAPS):
                    for c, (cs, kw) in enumerate(chunks_of(192)):
                        for m, (ms, mw) in enumerate(ms_list):
                            mms.append((ti, dy, dx, c, cs, kw, m, ms, mw))
                return i_lo, B, j_lo, mms

            # prologue part of step 0's z0/ctx accumulation
            def emit_z0_pre(t1):
                """seeds + f-part + (later) ctx-part for z0 of step t1 -> zc[t1%2][0:5]"""
                s1 = t1 % 2
                i_lo, B, j_lo = step_geom(t1)
                for m, (ms, mw) in enumerate(chunks_of(640)):
                    emit_seed2(zc[s1], m, brow[0][0:1, ms:ms + mw], mw, B)
                for k in range(3):
                    for m, (ms, mw) in enumerate(chunks_of(640)):
                        rhs = _ap(fm1, k * NPIX, i_lo * W + j_lo,
                                  [[3 * NPIX, 128], [W - 3, B]])
                        nc.tensor.matmul(zc[s1][0:mw, m, 0:B],
                                         W0fT[0:128, k, ms:ms + mw], rhs,
                                         start=False, stop=False, skip_group_check=True)

            def emit_z0_ctx(t1):
                s1 = t1 % 2
                i_lo, B, j_lo = step_geom(t1)
                for k in range(3):
                    for m, (ms, mw) in enumerate(chunks_of(640)):
                        nc.tensor.matmul(zc[s1][0:mw, m, 0:B],
                                         W0cT[0:128, k, ms:ms + mw],
                                         Xc[0:128, k, 0:B],
                                         start=False, stop=False, skip_group_check=True)

            def emit_old_mms(t1, geom, mms):
                i_lo, B, j_lo = geom
                s1 = t1 % 2
                for (ti, dy, dx, c, cs, kw, m, ms, mw) in mms:
                    rhs = ydiag_ap(Yimg, i_lo - dy, j_lo - dx, kw, c, B)
                    last = (ti == len(OLD_TAPS) - 1) and (c == 1)
                    nc.tensor.matmul(zc[s1][0:mw, 5 + m, 0:B],
                                     WdT[OLD_TAPS[ti]][0:kw, c, ms:ms + mw], rhs,
                                     start=False, stop=last, skip_group_check=True)

            # --- step 0 pre-work (its sources are all zero borders) ---
            g0 = emit_old_ctx(0)
            emit_old_mms(0, (g0[0], g0[1], g0[2]), g0[3])
            i_lo0, B0, j_lo0 = step_geom(0)
            nc.vector.tensor_copy(Xc[:, 0:3, 0:B0], zc[0][:, 5:8, 0:B0])
            emit_z0_pre(0)
            emit_z0_ctx(0)

            KCHW = {li: chunks_of(DIMS[li]) for li in range(1, 6)}
            MCHW = {li: chunks_of(DIMS[li + 1]) for li in range(0, 6)}

            for t in range(nsteps):
                s = t % 2
                s1 = (t + 1) % 2
                i_lo, B, j_lo = step_geom(t)
                have_next = t + 1 < nsteps
                if have_next:
                    i_lo1, B1, j_lo1 = step_geom(t + 1)

                # ---- fresh taps -> z0 (critical) ----
                for m, (ms, mw) in enumerate(MCHW[0]):
                    for f in range(2):
                        dy, dx = FRESH_TAPS[f]
                        for c, (cs, kw) in enumerate(chunks_of(192)):
                            rhs = ydiag_ap(Yimg, i_lo - dy, j_lo - dx, kw, c, B)
                            nc.tensor.matmul(zc[s][0:mw, m, 0:B],
                                             GT[f][0:kw, c, ms:ms + mw], rhs,
                                             start=False,
                                             stop=(f == 1 and c == 1),
                                             skip_group_check=True)

                # ---- lrelu z0 -> x1 (critical ACT) ----
                nc.scalar.activation(xs[1][:, 0:5, 0:B], zc[s][:, 0:5, 0:B],
                                     mybir.ActivationFunctionType.Lrelu, alpha=0.01)

                # old-ctx for t+1 (fillers)
                old_mms = []
                if have_next:
                    g = emit_old_ctx(t + 1)
                    old_mms = g[3]
                    geom1 = (g[0], g[1], g[2])

                # ---- MLP layers 1..5 ----
                for li in range(1, 6):
                    # fillers before the critical MMs of this layer
                    if li == 2 and have_next:
                        emit_old_mms(t + 1, geom1, old_mms[:33])
                    elif li == 3 and have_next:
                        emit_old_mms(t + 1, geom1, old_mms[33:])
                    elif li == 4 and have_next:
                        nc.vector.tensor_copy(Xc[:, 0:3, 0:B1], zc[s1][:, 5:8, 0:B1])
                        emit_z0_pre(t + 1)
                        pfill(10)
                    elif li == 5 and have_next:
                        pfill(14)

                    # seed + main MMs -> zs
                    for m, (ms, mw) in enumerate(MCHW[li]):
                        emit_seed2(zs, ZOFF[li] + m, brow[li][0:1, ms:ms + mw], mw, B)
                    kch = KCHW[li]
                    for m, (ms, mw) in enumerate(MCHW[li]):
                        for k, (ks, kw) in enumerate(kch):
                            nc.tensor.matmul(zs[0:mw, ZOFF[li] + m, 0:B],
                                             WT[li][0:kw, k, ms:ms + mw],
                                             xs[li][0:kw, k, 0:B],
                                             start=False, stop=(k == len(kch) - 1),
                                             skip_group_check=True)
                    if li < 5:
                        nch = len(MCHW[li])
                        nc.scalar.activation(xs[li + 1][:, 0:nch, 0:B],
                                             zs[:, ZOFF[li]:ZOFF[li] + nch, 0:B],
                                             mybir.ActivationFunctionType.Lrelu,
                                             alpha=0.01)

                # ---- Y = z5 + w_hat (critical DVE) ----
                off = (i_lo + 2) * WP + (j_lo + 2)
                ydst = _ap(Yimg, 0, off, [[2 * HP * WP, 128], [HP * WP, 2], [DIAG, B]])
                ywim = _ap(wimg, 0, off, [[2 * HP * WP, 128], [HP * WP, 2], [DIAG, B]])
                nc.vector.tensor_tensor(ydst, zs[:, 12:14, 0:B], ywim,
                                        mybir.AluOpType.add)
                # late filler: ctx->z0 for t+1 (needs Xc evict from ~2 hops ago)
                if have_next:
                    emit_z0_ctx(t + 1)

            # ================= EPILOGUE =================
            with tc.tile_pool(name="epi", bufs=1) as epi:
                Yimg32 = epi.tile([128, 2, NPIX], F32)
                src = _ap(Yimg, 0, 2 * WP + 2,
                          [[2 * HP * WP, 128], [HP * WP, 2], [WP, H], [1, W]])
                dst = _ap(Yimg32, 0, 0,
                          [[2 * NPIX, 128], [NPIX, 2], [W, H], [1, W]])
                nc.vector.tensor_copy(dst, src)
                ov = out.ap()[0]
                for ci, (cs, cw) in enumerate(chunks_of(192)):
                    nc.sync.dma_start(
                        ov[cs:cs + cw],
                        Yimg32[0:cw, ci, :].rearrange("p (h w) -> p h w", h=H))

    nc.compile()
    return nc


def emit_conv2_unit(nc, pps, cw2T, m2, fm1, b_c2, mi, r0, r1, k0, k1, ps=None):
    """Emit conv2 MMs [k0, k1) for out-chunk mi over rows [r0, r1);
    27 MMs per unit. MM index kk = k * 3 + si."""
    F32 = mybir.dt.float32
    nr = r1 - r0
    if ps is None:
        ps = pps.tile([128, 384], F32, tag="cps", name="cps")
    for kk in range(k0, k1):
        k, si = kk // 3, kk % 3
        ky, kx = k // 3, k % 3
        lhsT = _ap(cw2T, (mi * 3 + si) * 9 * 128, k * 128,
                   [[3 * 3 * 9 * 128, 128], [1, 128]])
        rhs = _ap(m2, si * 34 * 50, (ky + r0) * 50 + kx,
                  [[3 * 34 * 50, 128], [50, nr], [1, 48]])
        nc.tensor.matmul(ps[:, 0:nr * 48], lhsT, rhs,
                         start=(kk == 0), stop=(kk == 26), skip_group_check=True)
    if k1 == 27:
        nc.scalar.activation(fm1[:, mi, r0 * 48:r1 * 48], ps[:, 0:nr * 48],
                             mybir.ActivationFunctionType.Identity,
                             bias=b_c2[:, mi][:, None], alpha=0.0)
    return ps


_NC_CACHE = {}


def kernel(**inputs):
    from concourse.bass_utils import run_bass_kernel_spmd
    key = "full"
    if key not in _NC_CACHE:
        _NC_CACHE[key] = build()
    nc = _NC_CACHE[key]
    in_map = {k: np.ascontiguousarray(np.asarray(v, dtype=np.float32))
              for k, v in inputs.items()}
    res = run_bass_kernel_spmd(nc, [in_map] * 8, core_ids=list(range(8)))
    return res.results[0]['out']


if __name__ == "__main__":
    t = build(nsteps=int(sys.argv[1]) if len(sys.argv) > 1 else NSTEPS)
    print("build ok")
    from concourse.timeline_sim import TimelineSim
    est = TimelineSim(t).simulate()
    print(f"HW exec time: {est:.0f} ns")


# revision 25
# speedup vs baseline: 4.9404x; 1.0563x over previous
"""Trainium2 Bass kernel for nn_BEE_Bin2Symbol (hyper-decoder + masked-conv
autoregressive MLP decoder).

Architecture (v2, latency-oriented):
- Sequential phase runs all GEMMs in [C_out-on-partitions, B-pixels-free]
  orientation (weights stationary as lhsT, activations moving): matmul cost
  scales with B<=16, transposes disappear, and each layer's nonlinearity is a
  single DVE scalar_tensor_tensor op  max(0.01*z, z)  reading PSUM directly.
- 140 slope-3 anti-diagonal wavefronts. Per step the critical chain is
  fresh-tap MMs -> lrelu(z0) -> W1 MMs -> lrelu(z1) -> ... -> z5 -> Y-add.
  Everything else (old-tap ctx GEMMs, f-part, bias seeds, phase-P conv
  streaming) is issued into the PE gaps between the chain's engine hops.
- Fresh taps (age-1) use precomposed G = W0c @ Wd_tap so they feed z0
  directly; old taps accumulate a ctx PSUM a step ahead, evicted by the
  scalar engine (ACT) off the critical path.
- All sequential-phase weights/activations in bf16 (PSUM accumulate f32).
- Hyper-decoder (2 stride-2 deconvs + 3x3 conv) in f32r as phase-decomposed
  GEMMs; conv2's last 3 row-blocks stream into the first ~15 wavefront steps.
"""
import sys, os
sys.path.insert(0, "/opt/trn_rl_repo")

import numpy as np

import concourse.bass as bass
import concourse.bacc as bacc
import concourse.mybir as mybir
import concourse.tile as tile
from concourse.masks import make_identity

F32 = mybir.dt.float32
F32R = mybir.dt.float32r
BF16 = mybir.dt.bfloat16

H, W = 32, 48
HP, WP = H + 4, W + 4            # padded image 36 x 52
NPIX = H * W
NSTEPS = 3 * (H - 1) + (W - 1) + 1   # 140
DIAG = WP - 3                    # 49: wavefront-diagonal stride in padded img

# taps (dy, dx): source pixel = (i-dy, j-dx); ctx_w index (ky,kx) = (2-dy, 2-dx)
TAPS = [(2, 2), (2, 1), (2, 0), (2, -1), (2, -2),
        (1, 2), (1, 1), (1, 0), (1, -1), (1, -2),
        (0, 1), (0, 2)]
FRESH_TAPS = [(1, -2), (0, 1)]                # age-1 taps (need step t-1)
OLD_TAPS = [d for d in TAPS if d not in FRESH_TAPS]

DIMS = [768, 640, 512, 384, 320, 256, 192]   # MLP dims; layer l: DIMS[l]->DIMS[l+1]
# z1..z5 chunk offsets inside the shared zs psum tile [128, 14, 16]
ZOFF = {1: 0, 2: 4, 3: 7, 4: 10, 5: 12}


def cdiv(a, b):
    return (a + b - 1) // b


def chunks_of(n, c=128):
    return [(s, min(c, n - s)) for s in range(0, n, c)]


def _ap(tile_ap, slot_off, elem_off, plist):
    """Build a custom AP into a [128, S, F]-shaped sbuf/psum tile."""
    base = tile_ap[:]
    return bass.AP(base.tensor, base.offset + slot_off + elem_off, plist)


def step_geom(t):
    i_lo = max(0, cdiv(t - (W - 1), 3))
    i_hi = min(H - 1, t // 3)
    return i_lo, i_hi - i_lo + 1, t - 3 * i_lo


def build(nsteps=NSTEPS):
    nc = bacc.Bacc()

    # ---------------- DRAM I/O ----------------
    di = {}
    di['z_hat'] = nc.dram_tensor('z_hat', [1, 192, 8, 12], F32, kind="ExternalInput")
    di['w_hat'] = nc.dram_tensor('w_hat', [1, 192, 32, 48], F32, kind="ExternalInput")
    di['hs_dw0'] = nc.dram_tensor('hs_dw0', [192, 192, 5, 5], F32, kind="ExternalInput")
    di['hs_db0'] = nc.dram_tensor('hs_db0', [192], F32, kind="ExternalInput")
    di['hs_dw1'] = nc.dram_tensor('hs_dw1', [192, 288, 5, 5], F32, kind="ExternalInput")
    di['hs_db1'] = nc.dram_tensor('hs_db1', [288], F32, kind="ExternalInput")
    di['hs_cw2'] = nc.dram_tensor('hs_cw2', [384, 288, 3, 3], F32, kind="ExternalInput")
    di['hs_cb2'] = nc.dram_tensor('hs_cb2', [384], F32, kind="ExternalInput")
    di['ctx_w'] = nc.dram_tensor('ctx_w', [384, 192, 5, 5], F32, kind="ExternalInput")
    di['ctx_b'] = nc.dram_tensor('ctx_b', [384], F32, kind="ExternalInput")
    for li in range(6):
        di[f'ep_w{li}'] = nc.dram_tensor(f'ep_w{li}', [DIMS[li + 1], DIMS[li]], F32,
                                         kind="ExternalInput")
        di[f'ep_b{li}'] = nc.dram_tensor(f'ep_b{li}', [DIMS[li + 1]], F32,
                                         kind="ExternalInput")
    out = nc.dram_tensor('out', [1, 192, 32, 48], F32, kind="ExternalOutput")

    with tile.TileContext(nc) as tc:
        with tc.tile_pool(name="pp", bufs=1) as pp, \
             tc.tile_pool(name="pps", bufs=1, space="PSUM") as pps:

            ident = pp.tile([128, 128], F32)
            make_identity(nc, ident[:])

            # ---------- persistent state ----------
            Yimg = pp.tile([128, 2, HP * WP], BF16)   # decoded image (padded)
            nc.vector.memset(Yimg[:], 0.0)
            wimg = pp.tile([128, 2, HP * WP], F32)    # w_hat residual (padded)
            nc.gpsimd.memset(wimg[:], 0.0)
            fm1 = pp.tile([128, 3, NPIX], BF16)       # conv2 output [384, 1536]
            m2 = pp.tile([128, 3, 34 * 50], BF16)     # deconv1 out (padded 34x50)
            nc.gpsimd.memset(m2[:], 0.0)
            cw2T = pp.tile([128, 3, 3, 9 * 128], BF16)  # [cin, mi, si, k*128+o]

            # transposed weights (bf16)
            W0fT = pp.tile([128, 3, 640], BF16)
            W0cT = pp.tile([128, 3, 640], BF16)
            WT = {}
            for li in range(1, 6):
                WT[li] = pp.tile([128, cdiv(DIMS[li], 128), DIMS[li + 1]], BF16,
                                 tag=f"W{li}T", name=f"W{li}T")
            GT = [pp.tile([128, 2, 640], BF16, tag=f"GT{k}", name=f"GT{k}") for k in range(2)]
            WdT = {}
            for d in OLD_TAPS:
                WdT[d] = pp.tile([128, 2, 384], BF16, tag=f"Wd{d[0]}_{d[1]}", name=f"Wd{d[0]}_{d[1]}")

            # bias rows (lhsT for K=1 seed matmuls)
            brow = [pp.tile([1, DIMS[li + 1]], BF16, tag=f"b{li}", name=f"b{li}") for li in range(6)]
            ctxb = pp.tile([1, 384], BF16)
            ones = pp.tile([1, 16], BF16)
            nc.vector.memset(ones[:], 1.0)

            # sequential-phase activations (single tiles; WAR handled by sems)
            xs = {li: pp.tile([128, cdiv(DIMS[li], 128), 16], BF16, tag=f"x{li}", name=f"x{li}")
                  for li in range(1, 6)}
            Xc = pp.tile([128, 3, 16], BF16)          # evicted old-ctx

            # persistent psum: z0+ctx ring (2 banks), mlp zs (1), conv stream (1)
            zc = [pps.tile([128, 8, 16], F32, tag=f"zc{s}", name=f"zc{s}") for s in range(2)]
            zs = pps.tile([128, 14, 16], F32, tag="zs")
            nc.vector.memset(zs[:], 0.0)
            nc.vector.memset(zc[0][:], 0.0)
            nc.vector.memset(zc[1][:], 0.0)

            # ============ PROLOGUE 1: hyper-decoder (DMA priority) ============
            with tc.tile_pool(name="proB", bufs=2) as pro, \
                 tc.tile_pool(name="prpsB", bufs=2, space="PSUM") as prps:

                # SP queue: deconv0 weights lead everything
                dw0t = []
                for mi, (ms, mw) in enumerate(chunks_of(192)):
                    dw = pro.tile([128, 2, 128 * 25], F32R, tag="dw", name="dw", bufs=3)
                    for ci, (cs, cww) in enumerate(chunks_of(192)):
                        nc.sync.dma_start(
                            dw[0:cww, ci, 0:mw * 25],
                            di['hs_dw0'].ap()[cs:cs + cww, ms:ms + mw]
                            .rearrange("c o kh kw -> c (o kh kw)").bitcast(F32R))
                    dw0t.append(dw)

                # Pool/SWDGE queue: small loads (zp first - deconv0 input)
                zp = pro.tile([128, 2, 10 * 14], F32R, tag="zp", bufs=1)
                nc.vector.memset(zp[:].bitcast(F32), 0.0)
                zv = di['z_hat'].ap()[0]
                for ci, (s, cw) in enumerate(chunks_of(192)):
                    dst = _ap(zp, ci * 140, 14 + 1, [[2 * 140, cw], [14, 8], [1, 12]])
                    nc.gpsimd.dma_start(dst, zv[s:s + cw].bitcast(F32R))

                def load_bias_col(name, n):
                    nch = cdiv(n, 128)
                    t = pp.tile([128, nch], F32, tag=f"b_{name}", name=f"b_{name}")
                    nc.vector.memset(t[:], 0.0)
                    for ci, (s, w_) in enumerate(chunks_of(n)):
                        nc.gpsimd.dma_start(t[0:w_, ci:ci + 1], di[name][s:s + w_][:, None])
                    return t
                b_d0 = load_bias_col('hs_db0', 192)
                b_d1 = load_bias_col('hs_db1', 288)
                b_c2 = load_bias_col('hs_cb2', 384)

                def load_brow(dst, dram, n):
                    st = pro.tile([1, 640], F32, tag="brs", name="brs", bufs=3)
                    nc.gpsimd.dma_start(st[0:1, 0:n], dram.ap()[None, :])
                    nc.vector.tensor_copy(dst[0:1, 0:n], st[0:1, 0:n])
                for li in range(6):
                    load_brow(brow[li], di[f'ep_b{li}'], DIMS[li + 1])
                load_brow(ctxb, di['ctx_b'], 384)

                whv = di['w_hat'].ap()[0]
                for ci, (s, cw) in enumerate(chunks_of(192)):
                    dst = _ap(wimg, ci * HP * WP, 2 * WP + 2,
                              [[2 * HP * WP, cw], [WP, H], [1, W]])
                    nc.gpsimd.dma_start(dst, whv[s:s + cw])

                m1 = pro.tile([128, 2, 18 * 26], F32R, tag="m1", bufs=1)
                nc.vector.memset(m1[:].bitcast(F32), 0.0)

                def deconv_chunk(inp_t, inp_hw, w_t, cin, mw, mi, out_t, bias_t):
                    hi, wi = inp_hw
                    ip_w = wi + 2
                    op_w = 2 * wi + 2
                    for py in range(2):
                        for px in range(2):
                            ps = prps.tile([128, 16 * 24], F32, tag="dps")
                            first = True
                            taps = [(u, v) for u in range(py, 5, 2) for v in range(px, 5, 2)]
                            for ti, (u, v) in enumerate(taps):
                                dy = (py + 2 - u) // 2
                                dx = (px + 2 - v) // 2
                                for ci, (cs, cww) in enumerate(chunks_of(cin)):
                                    lhsT = _ap(w_t, ci * 128 * 25, u * 5 + v,
                                               [[2 * 128 * 25, cww], [25, mw]])
                                    rhs = _ap(inp_t, ci * (hi + 2) * ip_w,
                                              (1 + dy) * ip_w + (1 + dx),
                                              [[2 * (hi + 2) * ip_w, cww], [ip_w, hi], [1, wi]])
                                    last = (ti == len(taps) - 1) and (ci == len(chunks_of(cin)) - 1)
                                    nc.tensor.matmul(ps[0:mw, 0:hi * wi], lhsT, rhs,
                                                     start=first, stop=last,
                                                     skip_group_check=True)
                                    first = False
                            dst = _ap(out_t, mi * (2 * hi + 2) * op_w,
                                      (py + 1) * op_w + (px + 1),
                                      [[out_t.shape[1] * (2 * hi + 2) * op_w, mw],
                                       [2 * op_w, hi], [2, wi]])
                            nc.scalar.activation(
                                dst, ps[0:mw, 0:hi * wi].rearrange("p (a b) -> p a b", a=hi),
                                mybir.ActivationFunctionType.Lrelu,
                                bias=bias_t[0:mw, mi][:, None], alpha=0.01)

                # ACT queue: dw1 (so SP-queue dw rotation can't block it)
                dw1t = []
                for mi, (ms, mw) in enumerate(chunks_of(288)):
                    dw = pro.tile([128, 2, 128 * 25], F32R, tag="dw", name="dw1", bufs=3)
                    for ci, (cs, cww) in enumerate(chunks_of(192)):
                        nc.scalar.dma_start(
                            dw[0:cww, ci, 0:mw * 25],
                            di['hs_dw1'].ap()[cs:cs + cww, ms:ms + mw]
                            .rearrange("c o kh kw -> c (o kh kw)").bitcast(F32R))
                    dw1t.append(dw)

                # deconv0: z[192,8,12] -> m1[192,16,24]
                for mi, (ms, mw) in enumerate(chunks_of(192)):
                    deconv_chunk(zp, (8, 12), dw0t[mi], 192, mw, mi, m1, b_d0)

                # deconv1: m1[192,16,24] -> m2[288,32,48]
                for mi, (ms, mw) in enumerate(chunks_of(288)):
                    deconv_chunk(m1, (16, 24), dw1t[mi], 192, mw, mi, m2, b_d1)

            # ============ PROLOGUE 2: weight transposes ============
            with tc.tile_pool(name="pro", bufs=2) as pro, \
                 tc.tile_pool(name="prps", bufs=2, space="PSUM") as prps:

                tp_count = [0]
                def evict(dst_ap, src_ap):
                    if tp_count[0] % 2 == 0:
                        nc.vector.tensor_copy(dst_ap, src_ap)
                    else:
                        nc.scalar.activation(dst_ap, src_ap,
                                             mybir.ActivationFunctionType.Copy)
                    tp_count[0] += 1

                # SP queue (free after dw0): ep_w0 then ctx_w
                wnat0 = pro.tile([128, 5, 768], F32, tag="wnat0", bufs=1)
                for mi, (ms, mw) in enumerate(chunks_of(640)):
                    nc.sync.dma_start(wnat0[0:mw, mi, 0:768], di['ep_w0'].ap()[ms:ms + mw])
                cwn = [pro.tile([128, 192 * 25], F32R, tag=f"cwn{mi}", bufs=1, name=f"cwn{mi}")
                       for mi in range(3)]
                for mi in range(3):
                    nc.sync.dma_start(
                        cwn[mi][:],
                        di['ctx_w'].ap()[mi * 128:(mi + 1) * 128]
                        .rearrange("o c kh kw -> o (c kh kw)").bitcast(F32R))

                W0cT32 = pro.tile([128, 3, 640], F32R, tag="w0c32", bufs=1)

                # ep_w0 -> W0fT / W0cT (+f32r ctx half), batched evicts
                for ci in range(6):
                    cww = 128
                    for mb in range(2):
                        mchunks = list(enumerate(chunks_of(640)))[mb * 4:(mb + 1) * 4]
                        if not mchunks:
                            continue
                        pt = prps.tile([128, 4, 128], F32, tag="tp")
                        for k, (mi, (ms, mw)) in enumerate(mchunks):
                            nc.tensor.transpose(pt[0:cww, k, 0:mw],
                                                wnat0[0:mw, mi, ci * 128:ci * 128 + cww],
                                                ident[0:mw, 0:mw])
                        ms0 = mchunks[0][1][0]
                        tw = sum(mw for _, (ms, mw) in mchunks)
                        span = pt[:, 0:len(mchunks), :].rearrange("p a b -> p (a b)")[0:cww, 0:tw]
                        if ci < 3:
                            evict(W0fT[0:cww, ci, ms0:ms0 + tw], span)
                        else:
                            nc.vector.tensor_copy(W0cT[0:cww, ci - 3, ms0:ms0 + tw], span)
                            nc.scalar.activation(W0cT32[0:cww, ci - 3, ms0:ms0 + tw], span,
                                                 mybir.ActivationFunctionType.Copy)

                # ctx_w -> WdT (old taps), batched 3 per evict
                for d in OLD_TAPS:
                    ky, kx = 2 - d[0], 2 - d[1]
                    for ci, (cs, cww) in enumerate(chunks_of(192)):
                        pt = prps.tile([128, 4, 128], F32, tag="tp")
                        for mi in range(3):
                            sap = _ap(cwn[mi], 0, cs * 25 + ky * 5 + kx,
                                      [[192 * 25, 128], [25, cww]]).bitcast(F32)
                            nc.tensor.transpose(pt[0:cww, mi, 0:128], sap, ident[:])
                        span = pt[:, 0:3, :].rearrange("p a b -> p (a b)")[0:cww, 0:384]
                        evict(WdT[d][0:cww, ci, 0:384], span)

                # Pool queue: ep_w1..5 (rotation stalls stay off HWDGE queues)
                def load_and_transpose(li):
                    n_out, n_in = DIMS[li + 1], DIMS[li]
                    wnat = pro.tile([128, 4, 640], F32, tag="wnatS", name="wnatS", bufs=2)
                    for mi, (ms, mw) in enumerate(chunks_of(n_out)):
                        nc.gpsimd.dma_start(wnat[0:mw, mi, 0:n_in],
                                            di[f'ep_w{li}'].ap()[ms:ms + mw])
                    for ci, (cs, cww) in enumerate(chunks_of(n_in)):
                        mchunks = list(enumerate(chunks_of(n_out)))
                        for mb in range(cdiv(len(mchunks), 4)):
                            mcb = mchunks[mb * 4:(mb + 1) * 4]
                            pt = prps.tile([128, 4, 128], F32, tag="tp")
                            for k, (mi, (ms, mw)) in enumerate(mcb):
                                nc.tensor.transpose(pt[0:cww, k, 0:mw],
                                                    wnat[0:mw, mi, cs:cs + cww],
                                                    ident[0:mw, 0:mw])
                            ms0 = mcb[0][1][0]
                            tw = sum(mw for _, (ms, mw) in mcb)
                            span = pt[:, 0:len(mcb), :].rearrange("p a b -> p (a b)")[0:cww, 0:tw]
                            evict(WT[li][0:cww, ci, ms0:ms0 + tw], span)
                for li in range(1, 6):
                    load_and_transpose(li)

                # GT[f] = (W0c @ Wd_tap)^T
                for f, d in enumerate(FRESH_TAPS):
                    ky, kx = 2 - d[0], 2 - d[1]
                    for mc, (cs, cww) in enumerate(chunks_of(192)):
                        for nh in range(2):
                            gp = prps.tile([128, 384], F32, tag="dps")
                            for ki in range(3):
                                lhsT = _ap(cwn[ki], 0, cs * 25 + ky * 5 + kx,
                                           [[192 * 25, 128], [25, cww]])
                                rhs = W0cT32[0:128, ki, nh * 320:(nh + 1) * 320]
                                nc.tensor.matmul(gp[0:cww, 0:320], lhsT, rhs,
                                                 start=(ki == 0), stop=(ki == 2),
                                                 skip_group_check=True)
                            evict(GT[f][0:cww, mc, nh * 320:(nh + 1) * 320],
                                  gp[0:cww, 0:320])

            # ============ PROLOGUE 3: conv2 weights ============
            with tc.tile_pool(name="proC", bufs=1) as pro, \
                 tc.tile_pool(name="prpsC", bufs=2, space="PSUM") as prps:
                cw2st = []
                for mi in range(3):
                    cw2s = pro.tile([128, 288 * 9], F32, tag=f"cw2s{mi}", name=f"cw2s{mi}")
                    nc.sync.dma_start(
                        cw2s[:],
                        di['hs_cw2'].ap()[mi * 128:(mi + 1) * 128]
                        .rearrange("o c kh kw -> o (c kh kw)"))
                    cw2st.append(cw2s)
                tp_count = [0]
                def evict(dst_ap, src_ap):
                    if tp_count[0] % 2 == 0:
                        nc.vector.tensor_copy(dst_ap, src_ap)
                    else:
                        nc.scalar.activation(dst_ap, src_ap,
                                             mybir.ActivationFunctionType.Copy)
                    tp_count[0] += 1
                for mi in range(3):
                    nc.vector.memset(cw2T[32:64, mi, 2, :], 0.0)
                    nc.gpsimd.memset(cw2T[64:128, mi, 2, :], 0.0)
                    for si, (ss, sw) in enumerate(chunks_of(288)):
                        for kb in range(3):
                            ks = list(range(9))[kb * 4:(kb + 1) * 4]
                            if not ks:
                                continue
                            pt = prps.tile([128, 4, 128], F32, tag="tp")
                            for kk, k in enumerate(ks):
                                sap = _ap(cw2st[mi], 0, ss * 9 + k, [[288 * 9, 128], [9, sw]])
                                nc.tensor.transpose(pt[0:sw, kk, 0:128], sap, ident[:])
                            span = pt[:, 0:len(ks), :].rearrange("p a b -> p (a b)")[0:sw, 0:len(ks) * 128]
                            evict(_ap(cw2T, (mi * 3 + si) * 9 * 128, ks[0] * 128,
                                      [[3 * 3 * 9 * 128, sw], [1, len(ks) * 128]]),
                                  span)

            # conv2 rows 0..1 upfront; rest streamed into the wavefront steps
            for mi in range(3):
                emit_conv2_unit(nc, pps, cw2T, m2, fm1, b_c2, mi, 0, 1, 0, 27)

            # ================= SEQUENTIAL PHASE =================
            # conv2 streaming: 2-row units (mi, rb), rows [2rb, 2rb+2)
            pf_units = [(mi, rb) for rb in range(1, 16) for mi in range(3)]
            pf_state = {"u": 0, "k": 0, "ps": None}

            def pfill(nmm):
                while nmm > 0 and pf_state["u"] < len(pf_units):
                    mi, rb = pf_units[pf_state["u"]]
                    take = min(nmm, 27 - pf_state["k"])
                    ps = emit_conv2_unit(nc, pps, cw2T, m2, fm1, b_c2, mi,
                                         2 * rb, 2 * rb + 2,
                                         pf_state["k"], pf_state["k"] + take,
                                         ps=pf_state["ps"])
                    pf_state["ps"] = ps
                    pf_state["k"] += take
                    nmm -= take
                    if pf_state["k"] == 27:
                        pf_state["u"] += 1
                        pf_state["k"] = 0
                        pf_state["ps"] = None

            def ydiag_ap(img, i0, j0, kw, c, B):
                """[kw, B] wavefront-diagonal AP into padded img tile chunk c."""
                off = (i0 + 2) * WP + (j0 + 2)
                return _ap(img, c * HP * WP, off, [[2 * HP * WP, kw], [DIAG, B]])

            def emit_seed2(pt, slot, brow_ap, mw, B):
                nc.tensor.matmul(pt[0:mw, slot, 0:B], brow_ap, ones[0:1, 0:B],
                                 start=True, stop=False, skip_group_check=True)

            def emit_old_ctx(t1):
                """ctx_b seed for step t1's ctx -> zc[t1%2][5:8]; return tap MM list."""
                s1 = t1 % 2
                i_lo, B, j_lo = step_geom(t1)
                ms_list = chunks_of(384)
                for m, (ms, mw) in enumerate(ms_list):
                    emit_seed2(zc[s1], 5 + m, ctxb[0:1, ms:ms + mw], mw, B)
                mms = []
                for ti, (dy, dx) in enumerate(OLD_TAPS):
                    for c, (cs, kw) in enumerate(chunks_of(192)):
                        for m, (ms, mw) in enumerate(ms_list):
                            mms.append((ti, dy, dx, c, cs, kw, m, ms, mw))
                return i_lo, B, j_lo, mms

            # prologue part of step 0's z0/ctx accumulation
            def emit_z0_pre(t1):
                """seeds + f-part + (later) ctx-part for z0 of step t1 -> zc[t1%2][0:5]"""
                s1 = t1 % 2
                i_lo, B, j_lo = step_geom(t1)
                for m, (ms, mw) in enumerate(chunks_of(640)):
                    emit_seed2(zc[s1], m, brow[0][0:1, ms:ms + mw], mw, B)
                for k in range(3):
                    for m, (ms, mw) in enumerate(chunks_of(640)):
                        rhs = _ap(fm1, k * NPIX, i_lo * W + j_lo,
                                  [[3 * NPIX, 128], [W - 3, B]])
                        nc.tensor.matmul(zc[s1][0:mw, m, 0:B],
                                         W0fT[0:128, k, ms:ms + mw], rhs,
                                         start=False, stop=False, skip_group_check=True)

            def emit_z0_ctx(t1):
                s1 = t1 % 2
                i_lo, B, j_lo = step_geom(t1)
                for k in range(3):
                    for m, (ms, mw) in enumerate(chunks_of(640)):
                        nc.tensor.matmul(zc[s1][0:mw, m, 0:B],
                                         W0cT[0:128, k, ms:ms + mw],
                                         Xc[0:128, k, 0:B],
                                         start=False, stop=False, skip_group_check=True)

            def emit_old_mms(t1, geom, mms):
                i_lo, B, j_lo = geom
                s1 = t1 % 2
                for (ti, dy, dx, c, cs, kw, m, ms, mw) in mms:
                    rhs = ydiag_ap(Yimg, i_lo - dy, j_lo - dx, kw, c, B)
                    last = (ti == len(OLD_TAPS) - 1) and (c == 1)
                    nc.tensor.matmul(zc[s1][0:mw, 5 + m, 0:B],
                                     WdT[OLD_TAPS[ti]][0:kw, c, ms:ms + mw], rhs,
                                     start=False, stop=last, skip_group_check=True)

            # --- step 0 pre-work (its sources are all zero borders) ---
            g0 = emit_old_ctx(0)
            emit_old_mms(0, (g0[0], g0[1], g0[2]), g0[3])
            i_lo0, B0, j_lo0 = step_geom(0)
            nc.vector.tensor_copy(Xc[:, 0:3, 0:B0], zc[0][:, 5:8, 0:B0])
            emit_z0_pre(0)
            emit_z0_ctx(0)

            KCHW = {li: chunks_of(DIMS[li]) for li in range(1, 6)}
            MCHW = {li: chunks_of(DIMS[li + 1]) for li in range(0, 6)}

            for t in range(nsteps):
                s = t % 2
                s1 = (t + 1) % 2
                i_lo, B, j_lo = step_geom(t)
                have_next = t + 1 < nsteps
                if have_next:
                    i_lo1, B1, j_lo1 = step_geom(t + 1)

                # ---- fresh taps -> z0 (critical) ----
                for m, (ms, mw) in enumerate(MCHW[0]):
                    for f in range(2):
                        dy, dx = FRESH_TAPS[f]
                        for c, (cs, kw) in enumerate(chunks_of(192)):
                            rhs = ydiag_ap(Yimg, i_lo - dy, j_lo - dx, kw, c, B)
                            nc.tensor.matmul(zc[s][0:mw, m, 0:B],
                                             GT[f][0:kw, c, ms:ms + mw], rhs,
                                             start=False,
                                             stop=(f == 1 and c == 1),
                                             skip_group_check=True)

                # ---- lrelu z0 -> x1 (critical ACT) ----
                nc.scalar.activation(xs[1][:, 0:5, 0:B], zc[s][:, 0:5, 0:B],
                                     mybir.ActivationFunctionType.Lrelu, alpha=0.01)

                # old-ctx for t+1 (fillers)
                old_mms = []
                if have_next:
                    g = emit_old_ctx(t + 1)
                    old_mms = g[3]
                    geom1 = (g[0], g[1], g[2])

                # ---- MLP layers 1..5 ----
                for li in range(1, 6):
                    # fillers before the critical MMs of this layer
                    if li == 2 and have_next:
                        emit_old_mms(t + 1, geom1, old_mms[:33])
                    elif li == 3 and have_next:
                        emit_old_mms(t + 1, geom1, old_mms[33:])
                    elif li == 4 and have_next:
                        nc.vector.tensor_copy(Xc[:, 0:3, 0:B1], zc[s1][:, 5:8, 0:B1])
                        emit_z0_pre(t + 1)
                        pfill(10)
                    elif li == 5 and have_next:
                        pfill(14)

                    # seed + main MMs -> zs
                    for m, (ms, mw) in enumerate(MCHW[li]):
                        emit_seed2(zs, ZOFF[li] + m, brow[li][0:1, ms:ms + mw], mw, B)
                    kch = KCHW[li]
                    for m, (ms, mw) in enumerate(MCHW[li]):
                        for k, (ks, kw) in enumerate(kch):
                            nc.tensor.matmul(zs[0:mw, ZOFF[li] + m, 0:B],
                                             WT[li][0:kw, k, ms:ms + mw],
                                             xs[li][0:kw, k, 0:B],
                                             start=False, stop=(k == len(kch) - 1),
                                             skip_group_check=True)
                    if li < 5:
                        nch = len(MCHW[li])
                        nc.scalar.activation(xs[li + 1][:, 0:nch, 0:B],
                                             zs[:, ZOFF[li]:ZOFF[li] + nch, 0:B],
                                             mybir.ActivationFunctionType.Lrelu,
                                             alpha=0.01)

                # ---- Y = z5 + w_hat (critical DVE) ----
                off = (i_lo + 2) * WP + (j_lo + 2)
                ydst = _ap(Yimg, 0, off, [[2 * HP * WP, 128], [HP * WP, 2], [DIAG, B]])
                ywim = _ap(wimg, 0, off, [[2 * HP * WP, 128], [HP * WP, 2], [DIAG, B]])
                nc.vector.tensor_tensor(ydst, zs[:, 12:14, 0:B], ywim,
                                        mybir.AluOpType.add)
                # late filler: ctx->z0 for t+1 (needs Xc evict from ~2 hops ago)
                if have_next:
                    emit_z0_ctx(t + 1)

            # ================= EPILOGUE =================
            with tc.tile_pool(name="epi", bufs=1) as epi:
                Yimg32 = epi.tile([128, 2, NPIX], F32)
                src = _ap(Yimg, 0, 2 * WP + 2,
                          [[2 * HP * WP, 128], [HP * WP, 2], [WP, H], [1, W]])
                dst = _ap(Yimg32, 0, 0,
                          [[2 * NPIX, 128], [NPIX, 2], [W, H], [1, W]])
                nc.vector.tensor_copy(dst, src)
                ov = out.ap()[0]
                for ci, (cs, cw) in enumerate(chunks_of(192)):
                    nc.sync.dma_start(
                        ov[cs:cs + cw],
                        Yimg32[0:cw, ci, :].rearrange("p (h w) -> p h w", h=H))

    nc.compile()
    return nc


def emit_conv2_unit(nc, pps, cw2T, m2, fm1, b_c2, mi, r0, r1, k0, k1, ps=None):
    """Emit conv2 MMs [k0, k1) for out-chunk mi over rows [r0, r1);
    27 MMs per unit. MM index kk = k * 3 + si."""
    F32 = mybir.dt.float32
    nr = r1 - r0
    if ps is None:
        ps = pps.tile([128, 384], F32, tag="cps", name="cps")
    for kk in range(k0, k1):
        k, si = kk // 3, kk % 3
        ky, kx = k // 3, k % 3
        lhsT = _ap(cw2T, (mi * 3 + si) * 9 * 128, k * 128,
                   [[3 * 3 * 9 * 128, 128], [1, 128]])
        rhs = _ap(m2, si * 34 * 50, (ky + r0) * 50 + kx,
                  [[3 * 34 * 50, 128], [50, nr], [1, 48]])
        nc.tensor.matmul(ps[:, 0:nr * 48], lhsT, rhs,
                         start=(kk == 0), stop=(kk == 26), skip_group_check=True)
    if k1 == 27:
        nc.scalar.activation(fm1[:, mi, r0 * 48:r1 * 48], ps[:, 0:nr * 48],
                             mybir.ActivationFunctionType.Identity,
                             bias=b_c2[:, mi][:, None], alpha=0.0)
    return ps


_NC_CACHE = {}


def kernel(**inputs):
    from concourse.bass_utils import run_bass_kernel_spmd
    key = "full"
    if key not in _NC_CACHE:
        _NC_CACHE[key] = build()
    nc = _NC_CACHE[key]
    in_map = {k: np.ascontiguousarray(np.asarray(v, dtype=np.float32))
              for k, v in inputs.items()}
    res = run_bass_kernel_spmd(nc, [in_map] * 8, core_ids=list(range(8)))
    return res.results[0]['out']


if __name__ == "__main__":
    t = build(nsteps=int(sys.argv[1]) if len(sys.argv) > 1 else NSTEPS)
    print("build ok")
    from concourse.timeline_sim import TimelineSim
    est = TimelineSim(t).simulate()
    print(f"HW exec time: {est:.0f} ns")
